# revision 13
# baseline (speedup 1.0000x reference)
"""BiLSTM-CRF forward loss on 8 TRN2 NeuronCores (Bass/Tile).

Sharding: data-parallel over batch (32 seqs -> 4 per core), params replicated.
Each core computes per-sequence CRF numerator, logZ and aux-CE partials; the
host combines them into the scalar loss (pure unsharding arithmetic).
"""
import sys

import numpy as np

try:
    import concourse  # noqa: F401
except ImportError:  # pragma: no cover
    sys.path.insert(0, "/opt/trn_rl_repo")

import ml_dtypes
from contextlib import ExitStack

import concourse.bass as bass
import concourse.bacc as bacc
import concourse.mybir as mybir
import concourse.tile as tile
from concourse.bass_utils import run_bass_kernel_spmd

F32 = mybir.dt.float32
BF16 = mybir.dt.bfloat16
I32 = mybir.dt.int32
AF = mybir.ActivationFunctionType
ALU = mybir.AluOpType
AX = mybir.AxisListType

B, S, E, H, T, V = 32, 256, 256, 512, 17, 50000
NC = 8
BL = B // NC          # 4 local sequences per core
TOK = BL * S          # 1024 local tokens, flat index = 256*b + t
G = 4 * H             # 2048 gate rows
GC = G // 128         # 16 gate chunks
KH = H // 128         # 4 hidden chunks
RENORM = 8            # CRF renorm period
NREN = (S - 1) // RENORM  # 31 renorm events (t = 8,16,...,248)

_CACHE = {}


def _build_nc(steps=S):
    nc = bacc.Bacc(None, target_bir_lowering=False, num_devices=NC)
    d = {}
    P = nc.declare_dram_parameter
    d["x_idx"] = P("x_idx", [128, TOK // 128], I32, isOutput=False)
    d["emb"] = P("emb", [V, E], F32, isOutput=False)
    d["wih0T"] = P("wih0T", [2, 2, 128, G], BF16, isOutput=False)
    d["whh0T"] = P("whh0T", [2, 4, 128, G], BF16, isOutput=False)
    d["wih1T"] = P("wih1T", [2, 8, 128, G], BF16, isOutput=False)
    d["whh1T"] = P("whh1T", [2, 4, 128, G], BF16, isOutput=False)
    d["hwT"] = P("hwT", [2, 8, 128, 2 * H], BF16, isOutput=False)  # [t/h, k, p, o]
    d["fcwT"] = P("fcwT", [128, 8 * T], BF16, isOutput=False)  # [p, k*T]
    d["aux128"] = P("aux128", [128, 80], F32, isOutput=False)
    d["aux17"] = P("aux17", [T, 1120], F32, isOutput=False)
    d["aux1"] = P("aux1", [1, NREN * BL + TOK], F32, isOutput=False)
    d["msel"] = P("msel", [T, TOK], mybir.dt.uint8, isOutput=False)
    out_d = P("out", [4, BL], F32, isOutput=True)

    with tile.TileContext(nc) as tc, ExitStack() as ctx:
        pp = ctx.enter_context(tc.tile_pool(name="persist", bufs=1))
        wp = ctx.enter_context(tc.tile_pool(name="wts", bufs=1))
        sp = ctx.enter_context(tc.tile_pool(name="small", bufs=2))
        op = ctx.enter_context(tc.tile_pool(name="once", bufs=1))
        ps = ctx.enter_context(tc.tile_pool(name="psum", bufs=2, space="PSUM"))

        dma = nc.sync.dma_start

        # ---- static small loads -------------------------------------------------
        x_sb = pp.tile([128, TOK // 128], I32, tag="xidx")
        dma(x_sb[:], d["x_idx"][:])
        fcw_sb = pp.tile([128, 8, T], BF16, tag="fcw")
        dma(fcw_sb[:], d["fcwT"][:].rearrange("p (k t) -> p k t", k=8))
        aux128_sb = pp.tile([128, 80], F32, tag="aux128")
        dma(aux128_sb[:], d["aux128"][:])
        aux17_sb = pp.tile([T, 1120], F32, tag="aux17")
        dma(aux17_sb[:], d["aux17"][:])
        aux1_sb = pp.tile([1, NREN * BL + TOK], F32, tag="aux1")
        dma(aux1_sb[:], d["aux1"][:])
        msel_sb = pp.tile([T, BL, S], mybir.dt.uint8, tag="msel")
        dma(msel_sb[:], d["msel"][:].rearrange("t (b s) -> t b s", b=BL))

        def b0v(dd, c):
            return aux128_sb[:, dd * GC + c : dd * GC + c + 1]

        def b1v(dd, c):
            return aux128_sb[:, 32 + dd * GC + c : 32 + dd * GC + c + 1]

        def hwbv(w, c):
            return aux128_sb[:, 64 + 8 * w + c : 64 + 8 * w + c + 1]

        trans_sb = aux17_sb[:, 0:T]
        svec_sb = aux17_sb[:, T : T + 1]
        evec_sb = aux17_sb[:, T + 1 : T + 2]
        fcb_sb = aux17_sb[:, T + 2 : T + 3]
        oh_sb = aux17_sb[:, 20 : 20 + TOK].rearrange("t (b s) -> t b s", b=BL)
        cp_base = 20 + TOK
        s0e_sb = aux17_sb[:, cp_base + BL * T : cp_base + BL * T + 2 * BL]
        mren_sb = aux1_sb[:, 0 : NREN * BL].rearrange("o (k b) -> o k b", k=NREN)
        vm_sb = aux1_sb[:, NREN * BL :].rearrange("o (b s) -> o b s", b=BL)

        ones_t = pp.tile([T, 1], F32, tag="onesT")
        nc.vector.memset(ones_t[:], 1.0)
        ones_1t = pp.tile([1, T], F32, tag="ones1T")
        nc.vector.memset(ones_1t[:], 1.0)

        # ---- embedding gather + transpose --------------------------------------
        embX = pp.tile([128, TOK // 128, E], F32, tag="embX")
        for g in range(TOK // 128):
            nc.gpsimd.indirect_dma_start(
                out=embX[:, g, :],
                out_offset=None,
                in_=d["emb"][:],
                in_offset=bass.IndirectOffsetOnAxis(ap=x_sb[:, g : g + 1], axis=0),
            )
        embXbf = pp.tile([128, TOK // 128, E], BF16, tag="embXbf")
        for g in range(TOK // 128):
            nc.vector.tensor_copy(embXbf[:, g, :], embX[:, g, :])
        XT = pp.tile([128, E // 128, TOK], BF16, tag="XT")
        for k in range(E // 128):
            for g in range(TOK // 128):
                nc.sync.dma_start_transpose(
                    XT[:, k, bass.ts(g, 128)], embXbf[:, g, bass.ts(k, 128)]
                )

        # ---- L0 input GEMM ------------------------------------------------------
        wih0_sb = wp.tile([128, 2, 2, G], BF16, tag="wih")
        for dd in range(2):
            for k in range(2):
                dma(wih0_sb[:, dd, k, :], d["wih0T"][dd, k])
        gx = {}
        gx[("f", 0)] = pp.tile([128, GC, BL, S], BF16, tag="gxf", name="gx0f")
        gx[("b", 0)] = pp.tile([128, GC, BL, S], BF16, tag="gxb", name="gx0b")
        for dd, dn in enumerate("fb"):
            for c in range(GC):
                for b in range(BL):
                    pt = ps.tile([128, 256], F32, tag="mm")
                    for k in range(2):
                        nc.tensor.matmul(
                            pt[:],
                            wih0_sb[:, dd, k, bass.ts(c, 128)],
                            XT[:, k, bass.ts(b, 256)],
                            start=(k == 0),
                            stop=(k == 1),
                        )
                    nc.vector.tensor_scalar(
                        out=gx[(dn, 0)][:, c, b, :],
                        in0=pt[:],
                        scalar1=b0v(dd, c),
                        scalar2=None,
                        op0=ALU.add,
                    )

        # ---- recurrences --------------------------------------------------------
        whh_sb = {
            "f": wp.tile([128, 4, G], BF16, tag="whhf", name="whhf"),
            "b": wp.tile([128, 4, G], BF16, tag="whhb", name="whhb"),
        }
        for dd, dn in enumerate("fb"):
            for k in range(4):
                dma(whh_sb[dn][:, k, :], d["whh0T"][dd, k])

        hist = {}

        def lstm_layer(layer, steps):
            h_f = pp.tile([128, KH, S + 1, BL], BF16, tag="hhf")
            h_b = pp.tile([128, KH, S + 1, BL], BF16, tag="hhb")
            hist[(layer, "f")] = h_f
            hist[(layer, "b")] = h_b
            nc.vector.memset(h_f[:, :, 0, :], 0.0)
            nc.vector.memset(h_b[:, :, S, :], 0.0)
            cst = {}
            for par in range(2):
                cst[par] = pp.tile(
                    [128, 2, KH, BL], F32, tag=f"c{par}", name=f"c{layer}{par}"
                )
            nc.vector.memset(cst[0][:], 0.0)
            for t in range(steps):
                # fused fwd+bwd step: one PSUM tile, one gate chain for both dirs
                pt = ps.tile([128, 2, GC, BL], F32, tag="rec")
                for hx, (dn, hh) in enumerate((("f", h_f), ("b", h_b))):
                    rs = t if dn == "f" else S - t
                    for c in range(GC):
                        for k in range(KH):
                            nc.tensor.matmul(
                                pt[:, hx, c, :],
                                whh_sb[dn][:, k, bass.ts(c, 128)],
                                hh[:, k, rs, :],
                                start=(k == 0),
                                stop=(k == KH - 1),
                            )
                tmp = sp.tile([128, 2, GC, BL], F32, tag="tmp")
                nc.vector.tensor_add(
                    tmp[:, 0], pt[:, 0], gx[("f", layer)][:, :, :, t]
                )
                nc.vector.tensor_add(
                    tmp[:, 1], pt[:, 1], gx[("b", layer)][:, :, :, S - 1 - t]
                )
                sig = sp.tile([128, 2, GC, BL], F32, tag="sig")
                nc.scalar.activation(sig[:, :, 0:8, :], tmp[:, :, 0:8, :], AF.Sigmoid)
                nc.scalar.activation(sig[:, :, 8:12, :], tmp[:, :, 8:12, :], AF.Tanh)
                nc.scalar.activation(sig[:, :, 12:16, :], tmp[:, :, 12:16, :], AF.Sigmoid)
                c_old = cst[t % 2]
                c_new = cst[1 - t % 2]
                ig = sp.tile([128, 2, KH, BL], F32, tag="ig")
                nc.vector.tensor_mul(ig[:], sig[:, :, 0:4, :], sig[:, :, 8:12, :])
                nc.vector.tensor_mul(c_new[:], sig[:, :, 4:8, :], c_old[:])
                nc.vector.tensor_add(c_new[:], c_new[:], ig[:])
                th = sp.tile([128, 2, KH, BL], F32, tag="th")
                nc.scalar.activation(th[:], c_new[:], AF.Tanh)
                nc.vector.tensor_mul(h_f[:, :, t + 1, :], sig[:, 0, 12:16, :], th[:, 0])
                nc.vector.tensor_mul(h_b[:, :, S - 1 - t, :], sig[:, 1, 12:16, :], th[:, 1])

        lstm_layer(0, steps)

        # ---- L1 input GEMM ------------------------------------------------------
        gx[("f", 1)] = pp.tile([128, GC, BL, S], BF16, tag="gxf", name="gx1f")
        gx[("b", 1)] = pp.tile([128, GC, BL, S], BF16, tag="gxb", name="gx1b")
        for dd, dn in enumerate("fb"):
            wih1_sb = wp.tile([128, 8, G], BF16, tag="wih")
            for k in range(8):
                dma(wih1_sb[:, k, :], d["wih1T"][dd, k])
            for c in range(GC):
                for b in range(BL):
                    pt = ps.tile([128, 256], F32, tag="mm")
                    for k in range(8):
                        rhs = (
                            hist[(0, "f")][:, k, 1 : S + 1, b]
                            if k < KH
                            else hist[(0, "b")][:, k - KH, 0:S, b]
                        )
                        nc.tensor.matmul(
                            pt[:],
                            wih1_sb[:, k, bass.ts(c, 128)],
                            rhs,
                            start=(k == 0),
                            stop=(k == 7),
                        )
                    nc.vector.tensor_scalar(
                        out=gx[(dn, 1)][:, c, b, :],
                        in0=pt[:],
                        scalar1=b1v(dd, c),
                        scalar2=None,
                        op0=ALU.add,
                    )

        for dd, dn in enumerate("fb"):
            whh_sb[dn] = wp.tile([128, 4, G], BF16, tag=f"whh{dn}", name=f"whh1{dn}")
            for k in range(4):
                dma(whh_sb[dn][:, k, :], d["whh1T"][dd, k])
        lstm_layer(1, steps)

        # ---- highway + fc -------------------------------------------------------
        hw_sb = wp.tile([128, 2, 8, 2 * H], BF16, tag="wih")
        for w in range(2):
            for k in range(8):
                dma(hw_sb[:, w, k, :], d["hwT"][w, k])

        def x1_slice(k, b):
            if k < KH:
                return hist[(1, "f")][:, k, 1 : S + 1, b]
            return hist[(1, "b")][:, k - KH, 0:S, b]

        x2 = pp.tile([128, 8, TOK], BF16, tag="gxf")
        for c in range(8):
            for b in range(BL):
                ptt = ps.tile([128, 256], F32, tag="mm")
                pth = ps.tile([128, 256], F32, tag="mm")
                for k in range(8):
                    nc.tensor.matmul(
                        ptt[:], hw_sb[:, 0, k, bass.ts(c, 128)], x1_slice(k, b),
                        start=(k == 0), stop=(k == 7),
                    )
                for k in range(8):
                    nc.tensor.matmul(
                        pth[:], hw_sb[:, 1, k, bass.ts(c, 128)], x1_slice(k, b),
                        start=(k == 0), stop=(k == 7),
                    )
                tg = sp.tile([128, 256], F32, tag="tg")
                nc.scalar.activation(tg[:], ptt[:], AF.Sigmoid, bias=hwbv(0, c))
                rl = sp.tile([128, 256], F32, tag="rl")
                nc.scalar.activation(rl[:], pth[:], AF.Relu, bias=hwbv(1, c))
                dd_ = sp.tile([128, 256], F32, tag="dd")
                nc.vector.tensor_sub(dd_[:], rl[:], x1_slice(c, b))
                nc.vector.tensor_mul(dd_[:], tg[:], dd_[:])
                nc.vector.tensor_add(x2[:, c, bass.ts(b, 256)], dd_[:], x1_slice(c, b))

        logits = pp.tile([T, BL, S], F32, tag="embX")
        for b in range(BL):
            pt = ps.tile([128, 256], F32, tag="mm")
            for k in range(8):
                nc.tensor.matmul(
                    pt[:T, :], fcw_sb[:, k, :], x2[:, k, bass.ts(b, 256)],
                    start=(k == 0), stop=(k == 7),
                )
            nc.scalar.activation(logits[:, b, :], pt[:T, :], AF.Identity, bias=fcb_sb)

        # ---- CRF ---------------------------------------------------------------
        expEm = pp.tile([T, BL, S], F32, tag="XT")
        nc.scalar.activation(expEm[:], logits[:], AF.Exp)
        expT = pp.tile([T, T], F32, tag="expT")
        nc.scalar.activation(expT[:], trans_sb, AF.Exp)
        expS = pp.tile([T, 1], F32, tag="expS")
        nc.scalar.activation(expS[:], svec_sb, AF.Exp)
        expE = pp.tile([T, 1], F32, tag="expE")
        nc.scalar.activation(expE[:], evec_sb, AF.Exp)

        afin = pp.tile([T, BL], F32, tag="afin")
        lacc = {0: pp.tile([1, BL], F32, tag="lacc0", name="lacc0"), 1: pp.tile([1, BL], F32, tag="lacc1", name="lacc1")}
        nc.vector.memset(lacc[0][:], 0.0)
        ap = ctx.enter_context(tc.tile_pool(name="crf", bufs=4))

        A = ap.tile([T, BL], F32, tag="A")
        nc.vector.tensor_scalar(
            out=A[:], in0=expEm[:, :, 0], scalar1=expS[:, 0:1], scalar2=None, op0=ALU.mult
        )
        nren_seen = 0
        for t in range(1, steps):
            pt = ps.tile([128, BL], F32, tag="mm")
            nc.tensor.matmul(pt[:T, :], expT[:], A[:], start=True, stop=True)
            A = ap.tile([T, BL], F32, tag="A")
            nc.vector.tensor_mul(A[:], pt[:T, :], expEm[:, :, t])
            if t % RENORM == 0:
                psS = ps.tile([1, 512], F32, tag="small")
                nc.tensor.matmul(psS[:, :BL], ones_t[:], A[:], start=True, stop=True)
                Ssb = ap.tile([1, BL], F32, tag="Ssb")
                nc.vector.tensor_copy(Ssb[:], psS[:, :BL])
                Sr = ap.tile([1, BL], F32, tag="Sr")
                nc.vector.reciprocal(Sr[:], Ssb[:])
                pB = ps.tile([128, BL], F32, tag="mm")
                nc.tensor.matmul(pB[:T, :], ones_1t[:], Sr[:], start=True, stop=True)
                A2 = ap.tile([T, BL], F32, tag="A")
                nc.vector.tensor_mul(A2[:], A[:], pB[:T, :])
                A = A2
                lnS = ap.tile([1, BL], F32, tag="lnS")
                nc.scalar.activation(lnS[:], Ssb[:], AF.Ln)
                nc.vector.tensor_mul(lnS[:], lnS[:], mren_sb[:, nren_seen, :])
                old, new = lacc[nren_seen % 2], lacc[1 - nren_seen % 2]
                nc.vector.tensor_add(new[:], old[:], lnS[:])
                nren_seen += 1
            if t >= min(S // 2 - 1, steps - 1):
                nc.vector.copy_predicated(afin[:], msel_sb[:, :, t], A[:])

        lacc_f = lacc[nren_seen % 2]
        # logZ = ln(sum_j afin*expE) + lacc
        ae = op.tile([T, BL], F32, tag="ae")
        nc.vector.tensor_scalar(
            out=ae[:], in0=afin[:], scalar1=expE[:, 0:1], scalar2=None, op0=ALU.mult
        )
        psZ = ps.tile([1, 512], F32, tag="small")
        nc.tensor.matmul(psZ[:, :BL], ones_t[:], ae[:], start=True, stop=True)
        logZ = sp.tile([1, BL], F32, tag="logZ")
        nc.scalar.activation(logZ[:], psZ[:, :BL], AF.Ln)
        nc.vector.tensor_add(logZ[:], logZ[:], lacc_f[:])

        # ---- numerator ----------------------------------------------------------
        emm = op.tile([T, BL, S], F32, tag="emm")
        nc.vector.tensor_mul(emm[:], logits[:], oh_sb)
        empart = sp.tile([T, BL], F32, tag="empart")
        nc.vector.reduce_sum(empart[:], emm[:], axis=AX.X)
        nv = sp.tile([T, BL], F32, tag="nv")
        nc.vector.tensor_scalar(
            out=nv[:], in0=s0e_sb[:, 0:BL], scalar1=svec_sb, scalar2=None,
            op0=ALU.mult,
        )
        ev = sp.tile([T, BL], F32, tag="ev")
        nc.vector.tensor_scalar(
            out=ev[:], in0=s0e_sb[:, BL : 2 * BL], scalar1=evec_sb, scalar2=None,
            op0=ALU.mult,
        )
        nc.vector.tensor_add(nv[:], nv[:], ev[:])
        nc.vector.tensor_add(nv[:], nv[:], empart[:])
        for b in range(BL):
            trp = op.tile([T, T], F32, tag="trp")
            nc.vector.tensor_mul(trp[:], aux17_sb[:, cp_base + T * b : cp_base + T * (b + 1)], trans_sb)
            trr = sp.tile([T, 1], F32, tag="trr")
            nc.vector.reduce_sum(trr[:], trp[:], axis=AX.X)
            nc.vector.tensor_add(nv[:, b : b + 1], nv[:, b : b + 1], trr[:])
        psN = ps.tile([1, 512], F32, tag="small")
        nc.tensor.matmul(psN[:, :BL], ones_t[:], nv[:], start=True, stop=True)
        num_sb = sp.tile([1, BL], F32, tag="num")
        nc.vector.tensor_copy(num_sb[:], psN[:, :BL])

        # ---- aux CE -------------------------------------------------------------
        psE = ps.tile([1, 512], F32, tag="small")
        lse = op.tile([1, BL, S], F32, tag="lse")
        for hlf in range(2):
            nc.tensor.matmul(
                psE[:, :512],
                ones_t[:],
                expEm[:, 2 * hlf : 2 * hlf + 2, :],
                start=True,
                stop=True,
            )
            nc.scalar.activation(
                lse[:, 2 * hlf : 2 * hlf + 2, :],
                psE[:].rearrange("o (b s) -> o b s", b=2),
                AF.Ln,
            )
        nc.vector.tensor_mul(lse[:], lse[:], vm_sb)
        lsum = sp.tile([1, BL], F32, tag="lsum")
        nc.vector.reduce_sum(lsum[:], lse[:], axis=AX.X)
        psM = ps.tile([1, 512], F32, tag="small")
        nc.tensor.matmul(psM[:, :BL], ones_t[:], empart[:], start=True, stop=True)
        aux_sb = sp.tile([1, BL], F32, tag="aux")
        nc.vector.tensor_sub(aux_sb[:], lsum[:], psM[:, :BL])

        dma(out_d[0:1, :], num_sb[:])
        dma(out_d[1:2, :], logZ[:])
        dma(out_d[2:3, :], aux_sb[:])

    nc.compile()
    return nc


def _prep_maps(inputs):
    bf = ml_dtypes.bfloat16
    x = np.asarray(inputs["x"]).astype(np.int32)
    tags = np.asarray(inputs["tags"]).astype(np.int32)
    emb = np.asarray(inputs["emb"], np.float32)

    def t2(w):  # (2, G, K) -> (2, K//128, 128, G)
        w = np.asarray(w, np.float32)
        K = w.shape[2]
        return np.ascontiguousarray(
            w.transpose(0, 2, 1).reshape(2, K // 128, 128, G)
        ).astype(bf)

    wih0 = t2(inputs["w_ih_l0"])
    whh0 = t2(inputs["w_hh_l0"])
    wih1 = t2(inputs["w_ih_l1"])
    whh1 = t2(inputs["w_hh_l1"])
    b0 = np.asarray(inputs["b_l0"], np.float32).reshape(2, GC, 128)
    b1 = np.asarray(inputs["b_l1"], np.float32).reshape(2, GC, 128)
    aux128 = np.zeros((128, 80), np.float32)
    aux128[:, 0:32] = b0.reshape(32, 128).T
    aux128[:, 32:64] = b1.reshape(32, 128).T
    hwT = np.stack(
        [
            np.asarray(inputs["hw_t_w"], np.float32).T.reshape(8, 128, 2 * H),
            np.asarray(inputs["hw_h_w"], np.float32).T.reshape(8, 128, 2 * H),
        ]
    ).astype(bf)
    aux128[:, 64:72] = np.asarray(inputs["hw_t_b"], np.float32).reshape(8, 128).T
    aux128[:, 72:80] = np.asarray(inputs["hw_h_b"], np.float32).reshape(8, 128).T
    fcwT = np.ascontiguousarray(
        np.asarray(inputs["fc_w"], np.float32).T.reshape(8, 128, T).transpose(1, 0, 2)
    ).reshape(128, 8 * T).astype(bf)
    trans = np.asarray(inputs["crf_trans"], np.float32)
    svec = np.asarray(inputs["crf_start"], np.float32)
    evec = np.asarray(inputs["crf_end"], np.float32)
    fcb = np.asarray(inputs["fc_b"], np.float32)

    valid = tags != 0
    lengths = (x != 0).sum(1)

    maps = []
    for c in range(NC):
        sl = slice(c * BL, (c + 1) * BL)
        xl, tl, vl, ll = x[sl], tags[sl], valid[sl], lengths[sl]
        flat = xl.reshape(-1)  # 256*b + t
        x_idx = np.ascontiguousarray(flat.reshape(TOK // 128, 128).T).astype(np.int32)
        jj = np.arange(T)
        oh = (tl[None, :, :] == jj[:, None, None]) & vl[None, :, :]
        oh_tags = oh.reshape(T, TOK).astype(np.float32)
        cp = np.zeros((T, BL, T), np.float32)
        for b in range(BL):
            for t in range(1, S):
                if vl[b, t]:
                    cp[tl[b, t - 1], b, tl[b, t]] += 1.0
        s0e = np.zeros((T, 2 * BL), np.float32)
        for b in range(BL):
            s0e[tl[b, 0], b] = 1.0
            s0e[tl[b, ll[b] - 1], BL + b] = 1.0
        msel = np.zeros((BL, S), np.float32)
        for b in range(BL):
            msel[b, ll[b] - 1] = 1.0
        msel = np.broadcast_to(msel.reshape(1, TOK), (T, TOK)).astype(np.uint8)
        mren = np.zeros((NREN, BL), np.float32)
        for k in range(NREN):
            mren[k] = (RENORM * (k + 1) <= ll - 1).astype(np.float32)
        aux17 = np.zeros((T, 1120), np.float32)
        aux17[:, 0:T] = trans
        aux17[:, T] = svec
        aux17[:, T + 1] = evec
        aux17[:, T + 2] = fcb
        aux17[:, 20 : 20 + TOK] = oh_tags
        aux17[:, 20 + TOK : 20 + TOK + BL * T] = cp.transpose(0, 1, 2).reshape(T, BL * T)
        aux17[:, 20 + TOK + BL * T : 20 + TOK + BL * T + 2 * BL] = s0e
        aux1 = np.concatenate(
            [mren.reshape(-1), vl.reshape(-1).astype(np.float32)]
        ).reshape(1, -1)
        maps.append(
            dict(
                x_idx=x_idx,
                emb=emb,
                wih0T=wih0,
                whh0T=whh0,
                wih1T=wih1,
                whh1T=whh1,
                hwT=hwT,
                fcwT=fcwT,
                aux128=aux128,
                aux17=aux17,
                aux1=aux1,
                msel=msel,
            )
        )
    return maps, valid


TRACE = {}


def kernel(**inputs):
    if "nc" not in _CACHE:
        _CACHE["nc"] = _build_nc()
    nc = _CACHE["nc"]
    maps, valid = _prep_maps(inputs)
    kw = {}
    if TRACE.get("on"):
        kw = dict(trace=True, tmpdir=TRACE.get("dir"), trace_cores=[0])
    res = run_bass_kernel_spmd(nc, maps, list(range(NC)), **kw)
    TRACE["last"] = res
    outs = [res.results[i]["out"] for i in range(NC)]
    num = np.concatenate([o[0] for o in outs])
    logZ = np.concatenate([o[1] for o in outs])
    aux = np.concatenate([o[2] for o in outs])
    crf_loss = -np.mean(num - logZ, dtype=np.float32)
    aux_loss = np.float32(aux.sum()) / np.float32(max(valid.sum(), 1))
    return np.float32(crf_loss + np.float32(0.1) * aux_loss)


# revision 14
# speedup vs baseline: 1.0357x; 1.0357x over previous
"""BiLSTM-CRF forward loss on 8 TRN2 NeuronCores (Bass/Tile).

Sharding: data-parallel over batch (32 seqs -> 4 per core), params replicated.
Each core computes per-sequence CRF numerator, logZ and aux-CE partials; the
host combines them into the scalar loss (pure unsharding arithmetic).
"""
import sys

import numpy as np

try:
    import concourse  # noqa: F401
except ImportError:  # pragma: no cover
    sys.path.insert(0, "/opt/trn_rl_repo")

import ml_dtypes
from contextlib import ExitStack

import concourse.bass as bass
import concourse.bacc as bacc
import concourse.mybir as mybir
import concourse.tile as tile
from concourse.bass_utils import run_bass_kernel_spmd

F32 = mybir.dt.float32
BF16 = mybir.dt.bfloat16
I32 = mybir.dt.int32
AF = mybir.ActivationFunctionType
ALU = mybir.AluOpType
AX = mybir.AxisListType

B, S, E, H, T, V = 32, 256, 256, 512, 17, 50000
NC = 8
BL = B // NC          # 4 local sequences per core
TOK = BL * S          # 1024 local tokens, flat index = 256*b + t
G = 4 * H             # 2048 gate rows
GC = G // 128         # 16 gate chunks
KH = H // 128         # 4 hidden chunks
RENORM = 8            # CRF renorm period
NREN = (S - 1) // RENORM  # 31 renorm events (t = 8,16,...,248)

_CACHE = {}


def _build_nc(steps=S):
    nc = bacc.Bacc(None, target_bir_lowering=False, num_devices=NC)
    d = {}
    P = nc.declare_dram_parameter
    d["x_idx"] = P("x_idx", [128, TOK // 128], I32, isOutput=False)
    d["emb"] = P("emb", [V, E], F32, isOutput=False)
    d["wih0T"] = P("wih0T", [2, 2, 128, G], BF16, isOutput=False)
    d["whh0T"] = P("whh0T", [2, 4, 128, G], BF16, isOutput=False)
    d["wih1T"] = P("wih1T", [2, 8, 128, G], BF16, isOutput=False)
    d["whh1T"] = P("whh1T", [2, 4, 128, G], BF16, isOutput=False)
    d["hwT"] = P("hwT", [2, 8, 128, 2 * H], BF16, isOutput=False)  # [t/h, k, p, o]
    d["fcwT"] = P("fcwT", [128, 8 * T], BF16, isOutput=False)  # [p, k*T]
    d["aux128"] = P("aux128", [128, 80], F32, isOutput=False)
    d["aux17"] = P("aux17", [T, 1120], F32, isOutput=False)
    d["aux1"] = P("aux1", [1, NREN * BL + TOK], F32, isOutput=False)
    d["msel"] = P("msel", [T, TOK], mybir.dt.uint8, isOutput=False)
    out_d = P("out", [4, BL], F32, isOutput=True)

    with tile.TileContext(nc) as tc, ExitStack() as ctx:
        pp = ctx.enter_context(tc.tile_pool(name="persist", bufs=1))
        wp = ctx.enter_context(tc.tile_pool(name="wts", bufs=1))
        sp = ctx.enter_context(tc.tile_pool(name="small", bufs=2))
        op = ctx.enter_context(tc.tile_pool(name="once", bufs=1))
        ps = ctx.enter_context(tc.tile_pool(name="psum", bufs=2, space="PSUM"))

        dma = nc.sync.dma_start

        # ---- static small loads -------------------------------------------------
        x_sb = pp.tile([128, TOK // 128], I32, tag="xidx")
        dma(x_sb[:], d["x_idx"][:])
        fcw_sb = pp.tile([128, 8, T], BF16, tag="fcw")
        dma(fcw_sb[:], d["fcwT"][:].rearrange("p (k t) -> p k t", k=8))
        aux128_sb = pp.tile([128, 80], F32, tag="aux128")
        dma(aux128_sb[:], d["aux128"][:])
        aux17_sb = pp.tile([T, 1120], F32, tag="aux17")
        dma(aux17_sb[:], d["aux17"][:])
        aux1_sb = pp.tile([1, NREN * BL + TOK], F32, tag="aux1")
        dma(aux1_sb[:], d["aux1"][:])
        msel_sb = pp.tile([T, BL, S], mybir.dt.uint8, tag="msel")
        dma(msel_sb[:], d["msel"][:].rearrange("t (b s) -> t b s", b=BL))

        def b0v(dd, c):
            return aux128_sb[:, dd * GC + c : dd * GC + c + 1]

        def b1v(dd, c):
            return aux128_sb[:, 32 + dd * GC + c : 32 + dd * GC + c + 1]

        def hwbv(w, c):
            return aux128_sb[:, 64 + 8 * w + c : 64 + 8 * w + c + 1]

        trans_sb = aux17_sb[:, 0:T]
        svec_sb = aux17_sb[:, T : T + 1]
        evec_sb = aux17_sb[:, T + 1 : T + 2]
        fcb_sb = aux17_sb[:, T + 2 : T + 3]
        oh_sb = aux17_sb[:, 20 : 20 + TOK].rearrange("t (b s) -> t b s", b=BL)
        cp_base = 20 + TOK
        s0e_sb = aux17_sb[:, cp_base + BL * T : cp_base + BL * T + 2 * BL]
        mren_sb = aux1_sb[:, 0 : NREN * BL].rearrange("o (k b) -> o k b", k=NREN)
        vm_sb = aux1_sb[:, NREN * BL :].rearrange("o (b s) -> o b s", b=BL)

        ones_t = pp.tile([T, 1], F32, tag="onesT")
        nc.vector.memset(ones_t[:], 1.0)
        ones_1t = pp.tile([1, T], F32, tag="ones1T")
        nc.vector.memset(ones_1t[:], 1.0)

        # ---- embedding gather + transpose --------------------------------------
        embX = pp.tile([128, TOK // 128, E], F32, tag="embX")
        for g in range(TOK // 128):
            nc.gpsimd.indirect_dma_start(
                out=embX[:, g, :],
                out_offset=None,
                in_=d["emb"][:],
                in_offset=bass.IndirectOffsetOnAxis(ap=x_sb[:, g : g + 1], axis=0),
            )
        embXbf = pp.tile([128, TOK // 128, E], BF16, tag="embXbf")
        for g in range(TOK // 128):
            nc.vector.tensor_copy(embXbf[:, g, :], embX[:, g, :])
        XT = pp.tile([128, E // 128, TOK], BF16, tag="XT")
        for k in range(E // 128):
            for g in range(TOK // 128):
                nc.sync.dma_start_transpose(
                    XT[:, k, bass.ts(g, 128)], embXbf[:, g, bass.ts(k, 128)]
                )

        # ---- L0 input GEMM ------------------------------------------------------
        wih0_sb = wp.tile([128, 2, 2, G], BF16, tag="wih")
        for dd in range(2):
            for k in range(2):
                dma(wih0_sb[:, dd, k, :], d["wih0T"][dd, k])
        gx = {}
        gx[("f", 0)] = pp.tile([128, GC, BL, S], BF16, tag="gxf", name="gx0f")
        gx[("b", 0)] = pp.tile([128, GC, BL, S], BF16, tag="gxb", name="gx0b")
        for dd, dn in enumerate("fb"):
            for c in range(GC):
                for b in range(BL):
                    pt = ps.tile([128, 256], F32, tag="mm")
                    for k in range(2):
                        nc.tensor.matmul(
                            pt[:],
                            wih0_sb[:, dd, k, bass.ts(c, 128)],
                            XT[:, k, bass.ts(b, 256)],
                            start=(k == 0),
                            stop=(k == 1),
                        )
                    nc.vector.tensor_scalar(
                        out=gx[(dn, 0)][:, c, b, :],
                        in0=pt[:],
                        scalar1=b0v(dd, c),
                        scalar2=None,
                        op0=ALU.add,
                    )

        # ---- recurrences --------------------------------------------------------
        whh_sb = {
            "f": wp.tile([128, 4, G], BF16, tag="whhf", name="whhf"),
            "b": wp.tile([128, 4, G], BF16, tag="whhb", name="whhb"),
        }
        for dd, dn in enumerate("fb"):
            for k in range(4):
                dma(whh_sb[dn][:, k, :], d["whh0T"][dd, k])

        hist = {}

        def lstm_layer(layer, steps):
            h_f = pp.tile([128, KH, S + 1, BL], BF16, tag="hhf")
            h_b = pp.tile([128, KH, S + 1, BL], BF16, tag="hhb")
            hist[(layer, "f")] = h_f
            hist[(layer, "b")] = h_b
            nc.vector.memset(h_f[:, :, 0, :], 0.0)
            nc.vector.memset(h_b[:, :, S, :], 0.0)
            cst = {}
            for par in range(2):
                cst[par] = pp.tile(
                    [128, 2, KH, BL], F32, tag=f"c{par}", name=f"c{layer}{par}"
                )
            nc.vector.memset(cst[0][:], 0.0)
            for t in range(steps):
                # fused fwd+bwd gate chain; separate PSUM banks per direction
                pts = {}
                for hx, (dn, hh) in enumerate((("f", h_f), ("b", h_b))):
                    rs = t if dn == "f" else S - t
                    pt = ps.tile([128, GC, BL], F32, tag=f"rec{dn}")
                    pts[hx] = pt
                    for c in range(GC):
                        for k in range(KH):
                            nc.tensor.matmul(
                                pt[:, c, :],
                                whh_sb[dn][:, k, bass.ts(c, 128)],
                                hh[:, k, rs, :],
                                start=(k == 0),
                                stop=(k == KH - 1),
                            )
                tmp = sp.tile([128, 2, GC, BL], F32, tag="tmp")
                nc.vector.tensor_add(
                    tmp[:, 0], pts[0][:], gx[("f", layer)][:, :, :, t]
                )
                nc.vector.tensor_add(
                    tmp[:, 1], pts[1][:], gx[("b", layer)][:, :, :, S - 1 - t]
                )
                sig = sp.tile([128, 2, GC, BL], F32, tag="sig")
                nc.scalar.activation(sig[:, :, 0:8, :], tmp[:, :, 0:8, :], AF.Sigmoid)
                nc.scalar.activation(sig[:, :, 8:12, :], tmp[:, :, 8:12, :], AF.Tanh)
                nc.scalar.activation(sig[:, :, 12:16, :], tmp[:, :, 12:16, :], AF.Sigmoid)
                c_old = cst[t % 2]
                c_new = cst[1 - t % 2]
                ig = sp.tile([128, 2, KH, BL], F32, tag="ig")
                nc.vector.tensor_mul(ig[:], sig[:, :, 0:4, :], sig[:, :, 8:12, :])
                nc.vector.tensor_mul(c_new[:], sig[:, :, 4:8, :], c_old[:])
                nc.vector.tensor_add(c_new[:], c_new[:], ig[:])
                th = sp.tile([128, 2, KH, BL], F32, tag="th")
                nc.scalar.activation(th[:], c_new[:], AF.Tanh)
                nc.vector.tensor_mul(h_f[:, :, t + 1, :], sig[:, 0, 12:16, :], th[:, 0])
                nc.vector.tensor_mul(h_b[:, :, S - 1 - t, :], sig[:, 1, 12:16, :], th[:, 1])

        lstm_layer(0, steps)

        # ---- L1 input GEMM ------------------------------------------------------
        gx[("f", 1)] = pp.tile([128, GC, BL, S], BF16, tag="gxf", name="gx1f")
        gx[("b", 1)] = pp.tile([128, GC, BL, S], BF16, tag="gxb", name="gx1b")
        for dd, dn in enumerate("fb"):
            wih1_sb = wp.tile([128, 8, G], BF16, tag="wih")
            for k in range(8):
                dma(wih1_sb[:, k, :], d["wih1T"][dd, k])
            for c in range(GC):
                for b in range(BL):
                    pt = ps.tile([128, 256], F32, tag="mm")
                    for k in range(8):
                        rhs = (
                            hist[(0, "f")][:, k, 1 : S + 1, b]
                            if k < KH
                            else hist[(0, "b")][:, k - KH, 0:S, b]
                        )
                        nc.tensor.matmul(
                            pt[:],
                            wih1_sb[:, k, bass.ts(c, 128)],
                            rhs,
                            start=(k == 0),
                            stop=(k == 7),
                        )
                    nc.vector.tensor_scalar(
                        out=gx[(dn, 1)][:, c, b, :],
                        in0=pt[:],
                        scalar1=b1v(dd, c),
                        scalar2=None,
                        op0=ALU.add,
                    )

        for dd, dn in enumerate("fb"):
            whh_sb[dn] = wp.tile([128, 4, G], BF16, tag=f"whh{dn}", name=f"whh1{dn}")
            for k in range(4):
                dma(whh_sb[dn][:, k, :], d["whh1T"][dd, k])
        lstm_layer(1, steps)

        # ---- highway + fc -------------------------------------------------------
        hw_sb = wp.tile([128, 2, 8, 2 * H], BF16, tag="wih")
        for w in range(2):
            for k in range(8):
                dma(hw_sb[:, w, k, :], d["hwT"][w, k])

        def x1_slice(k, b):
            if k < KH:
                return hist[(1, "f")][:, k, 1 : S + 1, b]
            return hist[(1, "b")][:, k - KH, 0:S, b]

        x2 = pp.tile([128, 8, TOK], BF16, tag="gxf")
        for c in range(8):
            for b in range(BL):
                ptt = ps.tile([128, 256], F32, tag="mm")
                pth = ps.tile([128, 256], F32, tag="mm")
                for k in range(8):
                    nc.tensor.matmul(
                        ptt[:], hw_sb[:, 0, k, bass.ts(c, 128)], x1_slice(k, b),
                        start=(k == 0), stop=(k == 7),
                    )
                for k in range(8):
                    nc.tensor.matmul(
                        pth[:], hw_sb[:, 1, k, bass.ts(c, 128)], x1_slice(k, b),
                        start=(k == 0), stop=(k == 7),
                    )
                tg = sp.tile([128, 256], F32, tag="tg")
                nc.scalar.activation(tg[:], ptt[:], AF.Sigmoid, bias=hwbv(0, c))
                rl = sp.tile([128, 256], F32, tag="rl")
                nc.scalar.activation(rl[:], pth[:], AF.Relu, bias=hwbv(1, c))
                dd_ = sp.tile([128, 256], F32, tag="dd")
                nc.vector.tensor_sub(dd_[:], rl[:], x1_slice(c, b))
                nc.vector.tensor_mul(dd_[:], tg[:], dd_[:])
                nc.vector.tensor_add(x2[:, c, bass.ts(b, 256)], dd_[:], x1_slice(c, b))

        logits = pp.tile([T, BL, S], F32, tag="embX")
        for b in range(BL):
            pt = ps.tile([128, 256], F32, tag="mm")
            for k in range(8):
                nc.tensor.matmul(
                    pt[:T, :], fcw_sb[:, k, :], x2[:, k, bass.ts(b, 256)],
                    start=(k == 0), stop=(k == 7),
                )
            nc.scalar.activation(logits[:, b, :], pt[:T, :], AF.Identity, bias=fcb_sb)

        # ---- CRF ---------------------------------------------------------------
        expEm = pp.tile([T, BL, S], F32, tag="XT")
        nc.scalar.activation(expEm[:], logits[:], AF.Exp)
        expT = pp.tile([T, T], F32, tag="expT")
        nc.scalar.activation(expT[:], trans_sb, AF.Exp)
        expS = pp.tile([T, 1], F32, tag="expS")
        nc.scalar.activation(expS[:], svec_sb, AF.Exp)
        expE = pp.tile([T, 1], F32, tag="expE")
        nc.scalar.activation(expE[:], evec_sb, AF.Exp)

        afin = pp.tile([T, BL], F32, tag="afin")
        lacc = {0: pp.tile([1, BL], F32, tag="lacc0", name="lacc0"), 1: pp.tile([1, BL], F32, tag="lacc1", name="lacc1")}
        nc.vector.memset(lacc[0][:], 0.0)
        ap = ctx.enter_context(tc.tile_pool(name="crf", bufs=4))

        A = ap.tile([T, BL], F32, tag="A")
        nc.vector.tensor_scalar(
            out=A[:], in0=expEm[:, :, 0], scalar1=expS[:, 0:1], scalar2=None, op0=ALU.mult
        )
        nren_seen = 0
        for t in range(1, steps):
            pt = ps.tile([128, BL], F32, tag="mm")
            nc.tensor.matmul(pt[:T, :], expT[:], A[:], start=True, stop=True)
            A = ap.tile([T, BL], F32, tag="A")
            nc.vector.tensor_mul(A[:], pt[:T, :], expEm[:, :, t])
            if t % RENORM == 0:
                psS = ps.tile([1, 512], F32, tag="small")
                nc.tensor.matmul(psS[:, :BL], ones_t[:], A[:], start=True, stop=True)
                Ssb = ap.tile([1, BL], F32, tag="Ssb")
                nc.vector.tensor_copy(Ssb[:], psS[:, :BL])
                Sr = ap.tile([1, BL], F32, tag="Sr")
                nc.vector.reciprocal(Sr[:], Ssb[:])
                pB = ps.tile([128, BL], F32, tag="mm")
                nc.tensor.matmul(pB[:T, :], ones_1t[:], Sr[:], start=True, stop=True)
                A2 = ap.tile([T, BL], F32, tag="A")
                nc.vector.tensor_mul(A2[:], A[:], pB[:T, :])
                A = A2
                lnS = ap.tile([1, BL], F32, tag="lnS")
                nc.scalar.activation(lnS[:], Ssb[:], AF.Ln)
                nc.vector.tensor_mul(lnS[:], lnS[:], mren_sb[:, nren_seen, :])
                old, new = lacc[nren_seen % 2], lacc[1 - nren_seen % 2]
                nc.vector.tensor_add(new[:], old[:], lnS[:])
                nren_seen += 1
            if t >= min(S // 2 - 1, steps - 1):
                nc.vector.copy_predicated(afin[:], msel_sb[:, :, t], A[:])

        lacc_f = lacc[nren_seen % 2]
        # logZ = ln(sum_j afin*expE) + lacc
        ae = op.tile([T, BL], F32, tag="ae")
        nc.vector.tensor_scalar(
            out=ae[:], in0=afin[:], scalar1=expE[:, 0:1], scalar2=None, op0=ALU.mult
        )
        psZ = ps.tile([1, 512], F32, tag="small")
        nc.tensor.matmul(psZ[:, :BL], ones_t[:], ae[:], start=True, stop=True)
        logZ = sp.tile([1, BL], F32, tag="logZ")
        nc.scalar.activation(logZ[:], psZ[:, :BL], AF.Ln)
        nc.vector.tensor_add(logZ[:], logZ[:], lacc_f[:])

        # ---- numerator ----------------------------------------------------------
        emm = op.tile([T, BL, S], F32, tag="emm")
        nc.vector.tensor_mul(emm[:], logits[:], oh_sb)
        empart = sp.tile([T, BL], F32, tag="empart")
        nc.vector.reduce_sum(empart[:], emm[:], axis=AX.X)
        nv = sp.tile([T, BL], F32, tag="nv")
        nc.vector.tensor_scalar(
            out=nv[:], in0=s0e_sb[:, 0:BL], scalar1=svec_sb, scalar2=None,
            op0=ALU.mult,
        )
        ev = sp.tile([T, BL], F32, tag="ev")
        nc.vector.tensor_scalar(
            out=ev[:], in0=s0e_sb[:, BL : 2 * BL], scalar1=evec_sb, scalar2=None,
            op0=ALU.mult,
        )
        nc.vector.tensor_add(nv[:], nv[:], ev[:])
        nc.vector.tensor_add(nv[:], nv[:], empart[:])
        for b in range(BL):
            trp = op.tile([T, T], F32, tag="trp")
            nc.vector.tensor_mul(trp[:], aux17_sb[:, cp_base + T * b : cp_base + T * (b + 1)], trans_sb)
            trr = sp.tile([T, 1], F32, tag="trr")
            nc.vector.reduce_sum(trr[:], trp[:], axis=AX.X)
            nc.vector.tensor_add(nv[:, b : b + 1], nv[:, b : b + 1], trr[:])
        psN = ps.tile([1, 512], F32, tag="small")
        nc.tensor.matmul(psN[:, :BL], ones_t[:], nv[:], start=True, stop=True)
        num_sb = sp.tile([1, BL], F32, tag="num")
        nc.vector.tensor_copy(num_sb[:], psN[:, :BL])

        # ---- aux CE -------------------------------------------------------------
        psE = ps.tile([1, 512], F32, tag="small")
        lse = op.tile([1, BL, S], F32, tag="lse")
        for hlf in range(2):
            nc.tensor.matmul(
                psE[:, :512],
                ones_t[:],
                expEm[:, 2 * hlf : 2 * hlf + 2, :],
                start=True,
                stop=True,
            )
            nc.scalar.activation(
                lse[:, 2 * hlf : 2 * hlf + 2, :],
                psE[:].rearrange("o (b s) -> o b s", b=2),
                AF.Ln,
            )
        nc.vector.tensor_mul(lse[:], lse[:], vm_sb)
        lsum = sp.tile([1, BL], F32, tag="lsum")
        nc.vector.reduce_sum(lsum[:], lse[:], axis=AX.X)
        psM = ps.tile([1, 512], F32, tag="small")
        nc.tensor.matmul(psM[:, :BL], ones_t[:], empart[:], start=True, stop=True)
        aux_sb = sp.tile([1, BL], F32, tag="aux")
        nc.vector.tensor_sub(aux_sb[:], lsum[:], psM[:, :BL])

        dma(out_d[0:1, :], num_sb[:])
        dma(out_d[1:2, :], logZ[:])
        dma(out_d[2:3, :], aux_sb[:])

    nc.compile()
    return nc


def _prep_maps(inputs):
    bf = ml_dtypes.bfloat16
    x = np.asarray(inputs["x"]).astype(np.int32)
    tags = np.asarray(inputs["tags"]).astype(np.int32)
    emb = np.asarray(inputs["emb"], np.float32)

    def t2(w):  # (2, G, K) -> (2, K//128, 128, G)
        w = np.asarray(w, np.float32)
        K = w.shape[2]
        return np.ascontiguousarray(
            w.transpose(0, 2, 1).reshape(2, K // 128, 128, G)
        ).astype(bf)

    wih0 = t2(inputs["w_ih_l0"])
    whh0 = t2(inputs["w_hh_l0"])
    wih1 = t2(inputs["w_ih_l1"])
    whh1 = t2(inputs["w_hh_l1"])
    b0 = np.asarray(inputs["b_l0"], np.float32).reshape(2, GC, 128)
    b1 = np.asarray(inputs["b_l1"], np.float32).reshape(2, GC, 128)
    aux128 = np.zeros((128, 80), np.float32)
    aux128[:, 0:32] = b0.reshape(32, 128).T
    aux128[:, 32:64] = b1.reshape(32, 128).T
    hwT = np.stack(
        [
            np.asarray(inputs["hw_t_w"], np.float32).T.reshape(8, 128, 2 * H),
            np.asarray(inputs["hw_h_w"], np.float32).T.reshape(8, 128, 2 * H),
        ]
    ).astype(bf)
    aux128[:, 64:72] = np.asarray(inputs["hw_t_b"], np.float32).reshape(8, 128).T
    aux128[:, 72:80] = np.asarray(inputs["hw_h_b"], np.float32).reshape(8, 128).T
    fcwT = np.ascontiguousarray(
        np.asarray(inputs["fc_w"], np.float32).T.reshape(8, 128, T).transpose(1, 0, 2)
    ).reshape(128, 8 * T).astype(bf)
    trans = np.asarray(inputs["crf_trans"], np.float32)
    svec = np.asarray(inputs["crf_start"], np.float32)
    evec = np.asarray(inputs["crf_end"], np.float32)
    fcb = np.asarray(inputs["fc_b"], np.float32)

    valid = tags != 0
    lengths = (x != 0).sum(1)

    maps = []
    for c in range(NC):
        sl = slice(c * BL, (c + 1) * BL)
        xl, tl, vl, ll = x[sl], tags[sl], valid[sl], lengths[sl]
        flat = xl.reshape(-1)  # 256*b + t
        x_idx = np.ascontiguousarray(flat.reshape(TOK // 128, 128).T).astype(np.int32)
        jj = np.arange(T)
        oh = (tl[None, :, :] == jj[:, None, None]) & vl[None, :, :]
        oh_tags = oh.reshape(T, TOK).astype(np.float32)
        cp = np.zeros((T, BL, T), np.float32)
        for b in range(BL):
            for t in range(1, S):
                if vl[b, t]:
                    cp[tl[b, t - 1], b, tl[b, t]] += 1.0
        s0e = np.zeros((T, 2 * BL), np.float32)
        for b in range(BL):
            s0e[tl[b, 0], b] = 1.0
            s0e[tl[b, ll[b] - 1], BL + b] = 1.0
        msel = np.zeros((BL, S), np.float32)
        for b in range(BL):
            msel[b, ll[b] - 1] = 1.0
        msel = np.broadcast_to(msel.reshape(1, TOK), (T, TOK)).astype(np.uint8)
        mren = np.zeros((NREN, BL), np.float32)
        for k in range(NREN):
            mren[k] = (RENORM * (k + 1) <= ll - 1).astype(np.float32)
        aux17 = np.zeros((T, 1120), np.float32)
        aux17[:, 0:T] = trans
        aux17[:, T] = svec
        aux17[:, T + 1] = evec
        aux17[:, T + 2] = fcb
        aux17[:, 20 : 20 + TOK] = oh_tags
        aux17[:, 20 + TOK : 20 + TOK + BL * T] = cp.transpose(0, 1, 2).reshape(T, BL * T)
        aux17[:, 20 + TOK + BL * T : 20 + TOK + BL * T + 2 * BL] = s0e
        aux1 = np.concatenate(
            [mren.reshape(-1), vl.reshape(-1).astype(np.float32)]
        ).reshape(1, -1)
        maps.append(
            dict(
                x_idx=x_idx,
                emb=emb,
                wih0T=wih0,
                whh0T=whh0,
                wih1T=wih1,
                whh1T=whh1,
                hwT=hwT,
                fcwT=fcwT,
                aux128=aux128,
                aux17=aux17,
                aux1=aux1,
                msel=msel,
            )
        )
    return maps, valid


TRACE = {}


def kernel(**inputs):
    if "nc" not in _CACHE:
        _CACHE["nc"] = _build_nc()
    nc = _CACHE["nc"]
    maps, valid = _prep_maps(inputs)
    kw = {}
    if TRACE.get("on"):
        kw = dict(trace=True, tmpdir=TRACE.get("dir"), trace_cores=[0])
    res = run_bass_kernel_spmd(nc, maps, list(range(NC)), **kw)
    TRACE["last"] = res
    outs = [res.results[i]["out"] for i in range(NC)]
    num = np.concatenate([o[0] for o in outs])
    logZ = np.concatenate([o[1] for o in outs])
    aux = np.concatenate([o[2] for o in outs])
    crf_loss = -np.mean(num - logZ, dtype=np.float32)
    aux_loss = np.float32(aux.sum()) / np.float32(max(valid.sum(), 1))
    return np.float32(crf_loss + np.float32(0.1) * aux_loss)


# revision 15
# speedup vs baseline: 1.1684x; 1.1281x over previous
"""BiLSTM-CRF forward loss on 8 TRN2 NeuronCores (Bass/Tile).

Sharding: data-parallel over batch (32 seqs -> 4 per core), params replicated.
Each core computes per-sequence CRF numerator, logZ and aux-CE partials; the
host combines them into the scalar loss (pure unsharding arithmetic).
"""
import sys

import numpy as np

try:
    import concourse  # noqa: F401
except ImportError:  # pragma: no cover
    sys.path.insert(0, "/opt/trn_rl_repo")

import ml_dtypes
from contextlib import ExitStack

import concourse.bass as bass
import concourse.bacc as bacc
import concourse.mybir as mybir
import concourse.tile as tile
from concourse.bass_utils import run_bass_kernel_spmd

F32 = mybir.dt.float32
BF16 = mybir.dt.bfloat16
I32 = mybir.dt.int32
AF = mybir.ActivationFunctionType
ALU = mybir.AluOpType
AX = mybir.AxisListType

B, S, E, H, T, V = 32, 256, 256, 512, 17, 50000
NC = 8
BL = B // NC          # 4 local sequences per core
TOK = BL * S          # 1024 local tokens, flat index = 256*b + t
G = 4 * H             # 2048 gate rows
GC = G // 128         # 16 gate chunks
KH = H // 128         # 4 hidden chunks
RENORM = 8            # CRF renorm period
NREN = (S - 1) // RENORM  # 31 renorm events (t = 8,16,...,248)

_CACHE = {}


def _build_nc(steps=S):
    nc = bacc.Bacc(None, target_bir_lowering=False, num_devices=NC)
    d = {}
    P = nc.declare_dram_parameter
    d["x_idx"] = P("x_idx", [128, TOK // 128], I32, isOutput=False)
    d["emb"] = P("emb", [V, E], F32, isOutput=False)
    d["wih0T"] = P("wih0T", [2, 2, 128, G], BF16, isOutput=False)
    d["whh0T"] = P("whh0T", [2, 4, 128, G], BF16, isOutput=False)
    d["wih1T"] = P("wih1T", [2, 8, 128, G], BF16, isOutput=False)
    d["whh1T"] = P("whh1T", [2, 4, 128, G], BF16, isOutput=False)
    d["hwT"] = P("hwT", [2, 8, 128, 2 * H], BF16, isOutput=False)  # [t/h, k, p, o]
    d["fcwT"] = P("fcwT", [128, 8 * T], BF16, isOutput=False)  # [p, k*T]
    d["aux128"] = P("aux128", [128, 80], F32, isOutput=False)
    d["aux17"] = P("aux17", [T, 1120], F32, isOutput=False)
    d["aux1"] = P("aux1", [1, NREN * BL + TOK], F32, isOutput=False)
    d["msel"] = P("msel", [T, TOK], mybir.dt.uint8, isOutput=False)
    out_d = P("out", [4, BL], F32, isOutput=True)

    with tile.TileContext(nc) as tc, ExitStack() as ctx:
        pp = ctx.enter_context(tc.tile_pool(name="persist", bufs=1))
        wp = ctx.enter_context(tc.tile_pool(name="wts", bufs=1))
        sp = ctx.enter_context(tc.tile_pool(name="small", bufs=2))
        op = ctx.enter_context(tc.tile_pool(name="once", bufs=1))
        ps = ctx.enter_context(tc.tile_pool(name="psum", bufs=2, space="PSUM"))

        dma = nc.sync.dma_start

        # ---- static small loads -------------------------------------------------
        x_sb = pp.tile([128, TOK // 128], I32, tag="xidx")
        dma(x_sb[:], d["x_idx"][:])
        fcw_sb = pp.tile([128, 8, T], BF16, tag="fcw")
        dma(fcw_sb[:], d["fcwT"][:].rearrange("p (k t) -> p k t", k=8))
        aux128_sb = pp.tile([128, 80], F32, tag="aux128")
        dma(aux128_sb[:], d["aux128"][:])
        aux17_sb = pp.tile([T, 1120], F32, tag="aux17")
        dma(aux17_sb[:], d["aux17"][:])
        aux1_sb = pp.tile([1, NREN * BL + TOK], F32, tag="aux1")
        dma(aux1_sb[:], d["aux1"][:])
        msel_sb = pp.tile([T, BL, S], mybir.dt.uint8, tag="msel")
        dma(msel_sb[:], d["msel"][:].rearrange("t (b s) -> t b s", b=BL))

        def b0v(dd, c):
            return aux128_sb[:, dd * GC + c : dd * GC + c + 1]

        def b1v(dd, c):
            return aux128_sb[:, 32 + dd * GC + c : 32 + dd * GC + c + 1]

        def hwbv(w, c):
            return aux128_sb[:, 64 + 8 * w + c : 64 + 8 * w + c + 1]

        trans_sb = aux17_sb[:, 0:T]
        svec_sb = aux17_sb[:, T : T + 1]
        evec_sb = aux17_sb[:, T + 1 : T + 2]
        fcb_sb = aux17_sb[:, T + 2 : T + 3]
        oh_sb = aux17_sb[:, 20 : 20 + TOK].rearrange("t (b s) -> t b s", b=BL)
        cp_base = 20 + TOK
        s0e_sb = aux17_sb[:, cp_base + BL * T : cp_base + BL * T + 2 * BL]
        mren_sb = aux1_sb[:, 0 : NREN * BL].rearrange("o (k b) -> o k b", k=NREN)
        vm_sb = aux1_sb[:, NREN * BL :].rearrange("o (b s) -> o b s", b=BL)

        ones_t = pp.tile([T, 1], F32, tag="onesT")
        nc.vector.memset(ones_t[:], 1.0)
        ones_1t = pp.tile([1, T], F32, tag="ones1T")
        nc.vector.memset(ones_1t[:], 1.0)

        # ---- embedding gather + transpose --------------------------------------
        embX = pp.tile([128, TOK // 128, E], F32, tag="embX")
        for g in range(TOK // 128):
            nc.gpsimd.indirect_dma_start(
                out=embX[:, g, :],
                out_offset=None,
                in_=d["emb"][:],
                in_offset=bass.IndirectOffsetOnAxis(ap=x_sb[:, g : g + 1], axis=0),
            )
        embXbf = pp.tile([128, TOK // 128, E], BF16, tag="embXbf")
        for g in range(TOK // 128):
            nc.vector.tensor_copy(embXbf[:, g, :], embX[:, g, :])
        XT = pp.tile([128, E // 128, TOK], BF16, tag="XT")
        for k in range(E // 128):
            for g in range(TOK // 128):
                nc.sync.dma_start_transpose(
                    XT[:, k, bass.ts(g, 128)], embXbf[:, g, bass.ts(k, 128)]
                )

        # ---- L0 input GEMM ------------------------------------------------------
        wih0_sb = wp.tile([128, 2, 2, G], BF16, tag="wih")
        for dd in range(2):
            for k in range(2):
                dma(wih0_sb[:, dd, k, :], d["wih0T"][dd, k])
        gx = {}
        gx[("f", 0)] = pp.tile([128, GC, BL, S], BF16, tag="gxf", name="gx0f")
        gx[("b", 0)] = pp.tile([128, GC, BL, S], BF16, tag="gxb", name="gx0b")
        for dd, dn in enumerate("fb"):
            for c in range(GC):
                for b in range(BL):
                    pt = ps.tile([128, 256], F32, tag="mm")
                    for k in range(2):
                        nc.tensor.matmul(
                            pt[:],
                            wih0_sb[:, dd, k, bass.ts(c, 128)],
                            XT[:, k, bass.ts(b, 256)],
                            start=(k == 0),
                            stop=(k == 1),
                        )
                    nc.vector.tensor_scalar(
                        out=gx[(dn, 0)][:, c, b, :],
                        in0=pt[:],
                        scalar1=b0v(dd, c),
                        scalar2=None,
                        op0=ALU.add,
                    )

        # ---- recurrences --------------------------------------------------------
        whh_sb = {
            "f": wp.tile([128, 4, G], BF16, tag="whhf", name="whhf"),
            "b": wp.tile([128, 4, G], BF16, tag="whhb", name="whhb"),
        }
        for dd, dn in enumerate("fb"):
            for k in range(4):
                dma(whh_sb[dn][:, k, :], d["whh0T"][dd, k])

        hist = {}

        def lstm_layer(layer, steps):
            h_f = pp.tile([128, KH, S + 1, BL], BF16, tag="hhf")
            h_b = pp.tile([128, KH, S + 1, BL], BF16, tag="hhb")
            hist[(layer, "f")] = h_f
            hist[(layer, "b")] = h_b
            nc.vector.memset(h_f[:, :, 0, :], 0.0)
            nc.vector.memset(h_b[:, :, S, :], 0.0)
            cst = {}
            for dn in "fb":
                for par in range(2):
                    cst[(dn, par)] = pp.tile(
                        [128, KH, BL], F32, tag=f"c{dn}{par}", name=f"c{layer}{dn}{par}"
                    )
                nc.vector.memset(cst[(dn, 0)][:], 0.0)
            for t in range(steps):
                # per-dir chains; ifg/o PSUM split so the gate chain starts
                # while the same direction's o-gate matmuls are still running
                for dn, hh in (("f", h_f), ("b", h_b)):
                    pos = t if dn == "f" else S - 1 - t
                    rs = t if dn == "f" else pos + 1
                    ws = t + 1 if dn == "f" else pos
                    ptA = ps.tile([128, 12, BL], F32, tag=f"rec{dn}A", bufs=1)
                    ptB = ps.tile([128, 4, BL], F32, tag=f"rec{dn}B", bufs=1)
                    for c in range(GC):
                        dst = ptA[:, c, :] if c < 12 else ptB[:, c - 12, :]
                        for k in range(KH):
                            nc.tensor.matmul(
                                dst,
                                whh_sb[dn][:, k, bass.ts(c, 128)],
                                hh[:, k, rs, :],
                                start=(k == 0),
                                stop=(k == KH - 1),
                            )
                    tmp = sp.tile([128, GC, BL], F32, tag=f"tmp{dn}")
                    nc.vector.tensor_add(
                        tmp[:, 0:12, :], ptA[:], gx[(dn, layer)][:, 0:12, :, pos]
                    )
                    sig = sp.tile([128, GC, BL], F32, tag=f"sig{dn}")
                    nc.scalar.activation(sig[:, 0:8, :], tmp[:, 0:8, :], AF.Sigmoid)
                    nc.scalar.activation(sig[:, 8:12, :], tmp[:, 8:12, :], AF.Tanh)
                    c_old = cst[(dn, t % 2)]
                    c_new = cst[(dn, 1 - t % 2)]
                    ig = sp.tile([128, KH, BL], F32, tag=f"ig{dn}")
                    nc.vector.tensor_mul(ig[:], sig[:, 0:4, :], sig[:, 8:12, :])
                    nc.vector.tensor_mul(c_new[:], sig[:, 4:8, :], c_old[:])
                    nc.vector.tensor_add(c_new[:], c_new[:], ig[:])
                    th = sp.tile([128, KH, BL], F32, tag=f"th{dn}")
                    nc.scalar.activation(th[:], c_new[:], AF.Tanh)
                    nc.vector.tensor_add(
                        tmp[:, 12:16, :], ptB[:], gx[(dn, layer)][:, 12:16, :, pos]
                    )
                    nc.scalar.activation(sig[:, 12:16, :], tmp[:, 12:16, :], AF.Sigmoid)
                    nc.vector.tensor_mul(hh[:, :, ws, :], sig[:, 12:16, :], th[:])

        lstm_layer(0, steps)

        # ---- L1 input GEMM ------------------------------------------------------
        gx[("f", 1)] = pp.tile([128, GC, BL, S], BF16, tag="gxf", name="gx1f")
        gx[("b", 1)] = pp.tile([128, GC, BL, S], BF16, tag="gxb", name="gx1b")
        for dd, dn in enumerate("fb"):
            wih1_sb = wp.tile([128, 8, G], BF16, tag="wih")
            for k in range(8):
                dma(wih1_sb[:, k, :], d["wih1T"][dd, k])
            for c in range(GC):
                for b in range(BL):
                    pt = ps.tile([128, 256], F32, tag="mm")
                    for k in range(8):
                        rhs = (
                            hist[(0, "f")][:, k, 1 : S + 1, b]
                            if k < KH
                            else hist[(0, "b")][:, k - KH, 0:S, b]
                        )
                        nc.tensor.matmul(
                            pt[:],
                            wih1_sb[:, k, bass.ts(c, 128)],
                            rhs,
                            start=(k == 0),
                            stop=(k == 7),
                        )
                    nc.vector.tensor_scalar(
                        out=gx[(dn, 1)][:, c, b, :],
                        in0=pt[:],
                        scalar1=b1v(dd, c),
                        scalar2=None,
                        op0=ALU.add,
                    )

        for dd, dn in enumerate("fb"):
            whh_sb[dn] = wp.tile([128, 4, G], BF16, tag=f"whh{dn}", name=f"whh1{dn}")
            for k in range(4):
                dma(whh_sb[dn][:, k, :], d["whh1T"][dd, k])
        lstm_layer(1, steps)

        # ---- highway + fc -------------------------------------------------------
        hw_sb = wp.tile([128, 2, 8, 2 * H], BF16, tag="wih")
        for w in range(2):
            for k in range(8):
                dma(hw_sb[:, w, k, :], d["hwT"][w, k])

        def x1_slice(k, b):
            if k < KH:
                return hist[(1, "f")][:, k, 1 : S + 1, b]
            return hist[(1, "b")][:, k - KH, 0:S, b]

        x2 = pp.tile([128, 8, TOK], BF16, tag="gxf")
        for c in range(8):
            for b in range(BL):
                ptt = ps.tile([128, 256], F32, tag="mm")
                pth = ps.tile([128, 256], F32, tag="mm")
                for k in range(8):
                    nc.tensor.matmul(
                        ptt[:], hw_sb[:, 0, k, bass.ts(c, 128)], x1_slice(k, b),
                        start=(k == 0), stop=(k == 7),
                    )
                for k in range(8):
                    nc.tensor.matmul(
                        pth[:], hw_sb[:, 1, k, bass.ts(c, 128)], x1_slice(k, b),
                        start=(k == 0), stop=(k == 7),
                    )
                tg = sp.tile([128, 256], F32, tag="tg")
                nc.scalar.activation(tg[:], ptt[:], AF.Sigmoid, bias=hwbv(0, c))
                rl = sp.tile([128, 256], F32, tag="rl")
                nc.scalar.activation(rl[:], pth[:], AF.Relu, bias=hwbv(1, c))
                dd_ = sp.tile([128, 256], F32, tag="dd")
                nc.vector.tensor_sub(dd_[:], rl[:], x1_slice(c, b))
                nc.vector.tensor_mul(dd_[:], tg[:], dd_[:])
                nc.vector.tensor_add(x2[:, c, bass.ts(b, 256)], dd_[:], x1_slice(c, b))

        logits = pp.tile([T, BL, S], F32, tag="embX")
        for b in range(BL):
            pt = ps.tile([128, 256], F32, tag="mm")
            for k in range(8):
                nc.tensor.matmul(
                    pt[:T, :], fcw_sb[:, k, :], x2[:, k, bass.ts(b, 256)],
                    start=(k == 0), stop=(k == 7),
                )
            nc.scalar.activation(logits[:, b, :], pt[:T, :], AF.Identity, bias=fcb_sb)

        # ---- CRF ---------------------------------------------------------------
        expEm = pp.tile([T, BL, S], F32, tag="XT")
        nc.scalar.activation(expEm[:], logits[:], AF.Exp)
        expT = pp.tile([T, T], F32, tag="expT")
        nc.scalar.activation(expT[:], trans_sb, AF.Exp)
        expS = pp.tile([T, 1], F32, tag="expS")
        nc.scalar.activation(expS[:], svec_sb, AF.Exp)
        expE = pp.tile([T, 1], F32, tag="expE")
        nc.scalar.activation(expE[:], evec_sb, AF.Exp)

        afin = pp.tile([T, BL], F32, tag="afin")
        lacc = {0: pp.tile([1, BL], F32, tag="lacc0", name="lacc0"), 1: pp.tile([1, BL], F32, tag="lacc1", name="lacc1")}
        nc.vector.memset(lacc[0][:], 0.0)
        ap = ctx.enter_context(tc.tile_pool(name="crf", bufs=4))

        A = ap.tile([T, BL], F32, tag="A")
        nc.vector.tensor_scalar(
            out=A[:], in0=expEm[:, :, 0], scalar1=expS[:, 0:1], scalar2=None, op0=ALU.mult
        )
        nren_seen = 0
        for t in range(1, steps):
            pt = ps.tile([128, BL], F32, tag="mm")
            nc.tensor.matmul(pt[:T, :], expT[:], A[:], start=True, stop=True)
            A = ap.tile([T, BL], F32, tag="A")
            nc.vector.tensor_mul(A[:], pt[:T, :], expEm[:, :, t])
            if t % RENORM == 0:
                psS = ps.tile([1, 512], F32, tag="small")
                nc.tensor.matmul(psS[:, :BL], ones_t[:], A[:], start=True, stop=True)
                Ssb = ap.tile([1, BL], F32, tag="Ssb")
                nc.vector.tensor_copy(Ssb[:], psS[:, :BL])
                Sr = ap.tile([1, BL], F32, tag="Sr")
                nc.vector.reciprocal(Sr[:], Ssb[:])
                pB = ps.tile([128, BL], F32, tag="mm")
                nc.tensor.matmul(pB[:T, :], ones_1t[:], Sr[:], start=True, stop=True)
                A2 = ap.tile([T, BL], F32, tag="A")
                nc.vector.tensor_mul(A2[:], A[:], pB[:T, :])
                A = A2
                lnS = ap.tile([1, BL], F32, tag="lnS")
                nc.scalar.activation(lnS[:], Ssb[:], AF.Ln)
                nc.vector.tensor_mul(lnS[:], lnS[:], mren_sb[:, nren_seen, :])
                old, new = lacc[nren_seen % 2], lacc[1 - nren_seen % 2]
                nc.vector.tensor_add(new[:], old[:], lnS[:])
                nren_seen += 1
            if t >= min(S // 2 - 1, steps - 1):
                nc.vector.copy_predicated(afin[:], msel_sb[:, :, t], A[:])

        lacc_f = lacc[nren_seen % 2]
        # logZ = ln(sum_j afin*expE) + lacc
        ae = op.tile([T, BL], F32, tag="ae")
        nc.vector.tensor_scalar(
            out=ae[:], in0=afin[:], scalar1=expE[:, 0:1], scalar2=None, op0=ALU.mult
        )
        psZ = ps.tile([1, 512], F32, tag="small")
        nc.tensor.matmul(psZ[:, :BL], ones_t[:], ae[:], start=True, stop=True)
        logZ = sp.tile([1, BL], F32, tag="logZ")
        nc.scalar.activation(logZ[:], psZ[:, :BL], AF.Ln)
        nc.vector.tensor_add(logZ[:], logZ[:], lacc_f[:])

        # ---- numerator ----------------------------------------------------------
        emm = op.tile([T, BL, S], F32, tag="emm")
        nc.vector.tensor_mul(emm[:], logits[:], oh_sb)
        empart = sp.tile([T, BL], F32, tag="empart")
        nc.vector.reduce_sum(empart[:], emm[:], axis=AX.X)
        nv = sp.tile([T, BL], F32, tag="nv")
        nc.vector.tensor_scalar(
            out=nv[:], in0=s0e_sb[:, 0:BL], scalar1=svec_sb, scalar2=None,
            op0=ALU.mult,
        )
        ev = sp.tile([T, BL], F32, tag="ev")
        nc.vector.tensor_scalar(
            out=ev[:], in0=s0e_sb[:, BL : 2 * BL], scalar1=evec_sb, scalar2=None,
            op0=ALU.mult,
        )
        nc.vector.tensor_add(nv[:], nv[:], ev[:])
        nc.vector.tensor_add(nv[:], nv[:], empart[:])
        for b in range(BL):
            trp = op.tile([T, T], F32, tag="trp")
            nc.vector.tensor_mul(trp[:], aux17_sb[:, cp_base + T * b : cp_base + T * (b + 1)], trans_sb)
            trr = sp.tile([T, 1], F32, tag="trr")
            nc.vector.reduce_sum(trr[:], trp[:], axis=AX.X)
            nc.vector.tensor_add(nv[:, b : b + 1], nv[:, b : b + 1], trr[:])
        psN = ps.tile([1, 512], F32, tag="small")
        nc.tensor.matmul(psN[:, :BL], ones_t[:], nv[:], start=True, stop=True)
        num_sb = sp.tile([1, BL], F32, tag="num")
        nc.vector.tensor_copy(num_sb[:], psN[:, :BL])

        # ---- aux CE -------------------------------------------------------------
        psE = ps.tile([1, 512], F32, tag="small")
        lse = op.tile([1, BL, S], F32, tag="lse")
        for hlf in range(2):
            nc.tensor.matmul(
                psE[:, :512],
                ones_t[:],
                expEm[:, 2 * hlf : 2 * hlf + 2, :],
                start=True,
                stop=True,
            )
            nc.scalar.activation(
                lse[:, 2 * hlf : 2 * hlf + 2, :],
                psE[:].rearrange("o (b s) -> o b s", b=2),
                AF.Ln,
            )
        nc.vector.tensor_mul(lse[:], lse[:], vm_sb)
        lsum = sp.tile([1, BL], F32, tag="lsum")
        nc.vector.reduce_sum(lsum[:], lse[:], axis=AX.X)
        psM = ps.tile([1, 512], F32, tag="small")
        nc.tensor.matmul(psM[:, :BL], ones_t[:], empart[:], start=True, stop=True)
        aux_sb = sp.tile([1, BL], F32, tag="aux")
        nc.vector.tensor_sub(aux_sb[:], lsum[:], psM[:, :BL])

        dma(out_d[0:1, :], num_sb[:])
        dma(out_d[1:2, :], logZ[:])
        dma(out_d[2:3, :], aux_sb[:])

    nc.compile()
    return nc


def _prep_maps(inputs):
    bf = ml_dtypes.bfloat16
    x = np.asarray(inputs["x"]).astype(np.int32)
    tags = np.asarray(inputs["tags"]).astype(np.int32)
    emb = np.asarray(inputs["emb"], np.float32)

    def t2(w):  # (2, G, K) -> (2, K//128, 128, G)
        w = np.asarray(w, np.float32)
        K = w.shape[2]
        return np.ascontiguousarray(
            w.transpose(0, 2, 1).reshape(2, K // 128, 128, G)
        ).astype(bf)

    wih0 = t2(inputs["w_ih_l0"])
    whh0 = t2(inputs["w_hh_l0"])
    wih1 = t2(inputs["w_ih_l1"])
    whh1 = t2(inputs["w_hh_l1"])
    b0 = np.asarray(inputs["b_l0"], np.float32).reshape(2, GC, 128)
    b1 = np.asarray(inputs["b_l1"], np.float32).reshape(2, GC, 128)
    aux128 = np.zeros((128, 80), np.float32)
    aux128[:, 0:32] = b0.reshape(32, 128).T
    aux128[:, 32:64] = b1.reshape(32, 128).T
    hwT = np.stack(
        [
            np.asarray(inputs["hw_t_w"], np.float32).T.reshape(8, 128, 2 * H),
            np.asarray(inputs["hw_h_w"], np.float32).T.reshape(8, 128, 2 * H),
        ]
    ).astype(bf)
    aux128[:, 64:72] = np.asarray(inputs["hw_t_b"], np.float32).reshape(8, 128).T
    aux128[:, 72:80] = np.asarray(inputs["hw_h_b"], np.float32).reshape(8, 128).T
    fcwT = np.ascontiguousarray(
        np.asarray(inputs["fc_w"], np.float32).T.reshape(8, 128, T).transpose(1, 0, 2)
    ).reshape(128, 8 * T).astype(bf)
    trans = np.asarray(inputs["crf_trans"], np.float32)
    svec = np.asarray(inputs["crf_start"], np.float32)
    evec = np.asarray(inputs["crf_end"], np.float32)
    fcb = np.asarray(inputs["fc_b"], np.float32)

    valid = tags != 0
    lengths = (x != 0).sum(1)

    maps = []
    for c in range(NC):
        sl = slice(c * BL, (c + 1) * BL)
        xl, tl, vl, ll = x[sl], tags[sl], valid[sl], lengths[sl]
        flat = xl.reshape(-1)  # 256*b + t
        x_idx = np.ascontiguousarray(flat.reshape(TOK // 128, 128).T).astype(np.int32)
        jj = np.arange(T)
        oh = (tl[None, :, :] == jj[:, None, None]) & vl[None, :, :]
        oh_tags = oh.reshape(T, TOK).astype(np.float32)
        cp = np.zeros((T, BL, T), np.float32)
        for b in range(BL):
            for t in range(1, S):
                if vl[b, t]:
                    cp[tl[b, t - 1], b, tl[b, t]] += 1.0
        s0e = np.zeros((T, 2 * BL), np.float32)
        for b in range(BL):
            s0e[tl[b, 0], b] = 1.0
            s0e[tl[b, ll[b] - 1], BL + b] = 1.0
        msel = np.zeros((BL, S), np.float32)
        for b in range(BL):
            msel[b, ll[b] - 1] = 1.0
        msel = np.broadcast_to(msel.reshape(1, TOK), (T, TOK)).astype(np.uint8)
        mren = np.zeros((NREN, BL), np.float32)
        for k in range(NREN):
            mren[k] = (RENORM * (k + 1) <= ll - 1).astype(np.float32)
        aux17 = np.zeros((T, 1120), np.float32)
        aux17[:, 0:T] = trans
        aux17[:, T] = svec
        aux17[:, T + 1] = evec
        aux17[:, T + 2] = fcb
        aux17[:, 20 : 20 + TOK] = oh_tags
        aux17[:, 20 + TOK : 20 + TOK + BL * T] = cp.transpose(0, 1, 2).reshape(T, BL * T)
        aux17[:, 20 + TOK + BL * T : 20 + TOK + BL * T + 2 * BL] = s0e
        aux1 = np.concatenate(
            [mren.reshape(-1), vl.reshape(-1).astype(np.float32)]
        ).reshape(1, -1)
        maps.append(
            dict(
                x_idx=x_idx,
                emb=emb,
                wih0T=wih0,
                whh0T=whh0,
                wih1T=wih1,
                whh1T=whh1,
                hwT=hwT,
                fcwT=fcwT,
                aux128=aux128,
                aux17=aux17,
                aux1=aux1,
                msel=msel,
            )
        )
    return maps, valid


TRACE = {}


def kernel(**inputs):
    if "nc" not in _CACHE:
        _CACHE["nc"] = _build_nc()
    nc = _CACHE["nc"]
    maps, valid = _prep_maps(inputs)
    kw = {}
    if TRACE.get("on"):
        kw = dict(trace=True, tmpdir=TRACE.get("dir"), trace_cores=[0])
    res = run_bass_kernel_spmd(nc, maps, list(range(NC)), **kw)
    TRACE["last"] = res
    outs = [res.results[i]["out"] for i in range(NC)]
    num = np.concatenate([o[0] for o in outs])
    logZ = np.concatenate([o[1] for o in outs])
    aux = np.concatenate([o[2] for o in outs])
    crf_loss = -np.mean(num - logZ, dtype=np.float32)
    aux_loss = np.float32(aux.sum()) / np.float32(max(valid.sum(), 1))
    return np.float32(crf_loss + np.float32(0.1) * aux_loss)


# revision 16
# speedup vs baseline: 1.1694x; 1.0009x over previous
"""BiLSTM-CRF forward loss on 8 TRN2 NeuronCores (Bass/Tile).

Sharding: data-parallel over batch (32 seqs -> 4 per core), params replicated.
Each core computes per-sequence CRF numerator, logZ and aux-CE partials; the
host combines them into the scalar loss (pure unsharding arithmetic).
"""
import sys

import numpy as np

try:
    import concourse  # noqa: F401
except ImportError:  # pragma: no cover
    sys.path.insert(0, "/opt/trn_rl_repo")

import ml_dtypes
from contextlib import ExitStack

import concourse.bass as bass
import concourse.bacc as bacc
import concourse.mybir as mybir
import concourse.tile as tile
from concourse.bass_utils import run_bass_kernel_spmd

F32 = mybir.dt.float32
BF16 = mybir.dt.bfloat16
I32 = mybir.dt.int32
AF = mybir.ActivationFunctionType
ALU = mybir.AluOpType
AX = mybir.AxisListType

B, S, E, H, T, V = 32, 256, 256, 512, 17, 50000
NC = 8
BL = B // NC          # 4 local sequences per core
TOK = BL * S          # 1024 local tokens, flat index = 256*b + t
G = 4 * H             # 2048 gate rows
GC = G // 128         # 16 gate chunks
KH = H // 128         # 4 hidden chunks
RENORM = 8            # CRF renorm period
NREN = (S - 1) // RENORM  # 31 renorm events (t = 8,16,...,248)

_CACHE = {}


def _build_nc(steps=S):
    nc = bacc.Bacc(None, target_bir_lowering=False, num_devices=NC)
    d = {}
    P = nc.declare_dram_parameter
    d["x_idx"] = P("x_idx", [128, TOK // 128], I32, isOutput=False)
    d["emb"] = P("emb", [V, E], F32, isOutput=False)
    d["wih0T"] = P("wih0T", [2, 2, 128, G], BF16, isOutput=False)
    d["whh0T"] = P("whh0T", [2, 4, 128, G], BF16, isOutput=False)
    d["wih1T"] = P("wih1T", [2, 8, 128, G], BF16, isOutput=False)
    d["whh1T"] = P("whh1T", [2, 4, 128, G], BF16, isOutput=False)
    d["hwT"] = P("hwT", [2, 8, 128, 2 * H], BF16, isOutput=False)  # [t/h, k, p, o]
    d["fcwT"] = P("fcwT", [128, 8 * T], BF16, isOutput=False)  # [p, k*T]
    d["aux128"] = P("aux128", [128, 80], F32, isOutput=False)
    d["aux17"] = P("aux17", [T, 1120], F32, isOutput=False)
    d["aux1"] = P("aux1", [1, NREN * BL + TOK], F32, isOutput=False)
    d["msel"] = P("msel", [T, TOK], mybir.dt.uint8, isOutput=False)
    out_d = P("out", [4, BL], F32, isOutput=True)

    with tile.TileContext(nc) as tc, ExitStack() as ctx:
        pp = ctx.enter_context(tc.tile_pool(name="persist", bufs=1))
        wp = ctx.enter_context(tc.tile_pool(name="wts", bufs=1))
        sp = ctx.enter_context(tc.tile_pool(name="small", bufs=2))
        op = ctx.enter_context(tc.tile_pool(name="once", bufs=1))
        ps = ctx.enter_context(tc.tile_pool(name="psum", bufs=2, space="PSUM"))

        dma = nc.sync.dma_start

        # ---- static small loads -------------------------------------------------
        x_sb = pp.tile([128, TOK // 128], I32, tag="xidx")
        dma(x_sb[:], d["x_idx"][:])
        fcw_sb = pp.tile([128, 8, T], BF16, tag="fcw")
        dma(fcw_sb[:], d["fcwT"][:].rearrange("p (k t) -> p k t", k=8))
        aux128_sb = pp.tile([128, 80], F32, tag="aux128")
        dma(aux128_sb[:], d["aux128"][:])
        aux17_sb = pp.tile([T, 1120], F32, tag="aux17")
        dma(aux17_sb[:], d["aux17"][:])
        aux1_sb = pp.tile([1, NREN * BL + TOK], F32, tag="aux1")
        dma(aux1_sb[:], d["aux1"][:])
        msel_sb = pp.tile([T, BL, S], mybir.dt.uint8, tag="msel")
        dma(msel_sb[:], d["msel"][:].rearrange("t (b s) -> t b s", b=BL))

        def b0v(dd, c):
            return aux128_sb[:, dd * GC + c : dd * GC + c + 1]

        def b1v(dd, c):
            return aux128_sb[:, 32 + dd * GC + c : 32 + dd * GC + c + 1]

        def hwbv(w, c):
            return aux128_sb[:, 64 + 8 * w + c : 64 + 8 * w + c + 1]

        trans_sb = aux17_sb[:, 0:T]
        svec_sb = aux17_sb[:, T : T + 1]
        evec_sb = aux17_sb[:, T + 1 : T + 2]
        fcb_sb = aux17_sb[:, T + 2 : T + 3]
        oh_sb = aux17_sb[:, 20 : 20 + TOK].rearrange("t (b s) -> t b s", b=BL)
        cp_base = 20 + TOK
        s0e_sb = aux17_sb[:, cp_base + BL * T : cp_base + BL * T + 2 * BL]
        mren_sb = aux1_sb[:, 0 : NREN * BL].rearrange("o (k b) -> o k b", k=NREN)
        vm_sb = aux1_sb[:, NREN * BL :].rearrange("o (b s) -> o b s", b=BL)

        ones_t = pp.tile([T, 1], F32, tag="onesT")
        nc.vector.memset(ones_t[:], 1.0)
        ones_1t = pp.tile([1, T], F32, tag="ones1T")
        nc.vector.memset(ones_1t[:], 1.0)

        # ---- embedding gather + transpose --------------------------------------
        embX = pp.tile([128, TOK // 128, E], F32, tag="embX")
        for g in range(TOK // 128):
            nc.gpsimd.indirect_dma_start(
                out=embX[:, g, :],
                out_offset=None,
                in_=d["emb"][:],
                in_offset=bass.IndirectOffsetOnAxis(ap=x_sb[:, g : g + 1], axis=0),
            )
        embXbf = pp.tile([128, TOK // 128, E], BF16, tag="embXbf")
        for g in range(TOK // 128):
            nc.vector.tensor_copy(embXbf[:, g, :], embX[:, g, :])
        XT = pp.tile([128, E // 128, TOK], BF16, tag="XT")
        for k in range(E // 128):
            for g in range(TOK // 128):
                nc.sync.dma_start_transpose(
                    XT[:, k, bass.ts(g, 128)], embXbf[:, g, bass.ts(k, 128)]
                )

        # ---- L0 input GEMM ------------------------------------------------------
        wih0_sb = wp.tile([128, 2, 2, G], BF16, tag="wih")
        for dd in range(2):
            for k in range(2):
                dma(wih0_sb[:, dd, k, :], d["wih0T"][dd, k])
        gx = {}
        gx[("f", 0)] = pp.tile([128, GC, BL, S], BF16, tag="gxf", name="gx0f")
        gx[("b", 0)] = pp.tile([128, GC, BL, S], BF16, tag="gxb", name="gx0b")
        for dd, dn in enumerate("fb"):
            for c in range(GC):
                for b in range(BL):
                    pt = ps.tile([128, 256], F32, tag="mm")
                    for k in range(2):
                        nc.tensor.matmul(
                            pt[:],
                            wih0_sb[:, dd, k, bass.ts(c, 128)],
                            XT[:, k, bass.ts(b, 256)],
                            start=(k == 0),
                            stop=(k == 1),
                        )
                    nc.vector.tensor_scalar(
                        out=gx[(dn, 0)][:, c, b, :],
                        in0=pt[:],
                        scalar1=b0v(dd, c),
                        scalar2=None,
                        op0=ALU.add,
                    )

        # ---- recurrences --------------------------------------------------------
        whh_sb = {
            "f": wp.tile([128, 4, G], BF16, tag="whhf", name="whhf"),
            "b": wp.tile([128, 4, G], BF16, tag="whhb", name="whhb"),
        }
        for dd, dn in enumerate("fb"):
            for k in range(4):
                dma(whh_sb[dn][:, k, :], d["whh0T"][dd, k])

        hist = {}

        def lstm_layer(layer, steps):
            h_f = pp.tile([128, KH, S + 1, BL], BF16, tag="hhf")
            h_b = pp.tile([128, KH, S + 1, BL], BF16, tag="hhb")
            hist[(layer, "f")] = h_f
            hist[(layer, "b")] = h_b
            nc.vector.memset(h_f[:, :, 0, :], 0.0)
            nc.vector.memset(h_b[:, :, S, :], 0.0)
            cst = {}
            for dn in "fb":
                for par in range(2):
                    cst[(dn, par)] = pp.tile(
                        [128, KH, BL], F32, tag=f"c{dn}{par}", name=f"c{layer}{dn}{par}"
                    )
                nc.vector.memset(cst[(dn, 0)][:], 0.0)
            for t in range(steps):
                # per-dir chains; ifg/o PSUM split so the gate chain starts
                # while the same direction's o-gate matmuls are still running
                for dn, hh in (("f", h_f), ("b", h_b)):
                    pos = t if dn == "f" else S - 1 - t
                    rs = t if dn == "f" else pos + 1
                    ws = t + 1 if dn == "f" else pos
                    ptA = ps.tile([128, 12, BL], F32, tag=f"rec{dn}A", bufs=1)
                    ptB = ps.tile([128, 4, BL], F32, tag=f"rec{dn}B", bufs=1)
                    for c in range(GC):
                        dst = ptA[:, c, :] if c < 12 else ptB[:, c - 12, :]
                        for k in range(KH):
                            nc.tensor.matmul(
                                dst,
                                whh_sb[dn][:, k, bass.ts(c, 128)],
                                hh[:, k, rs, :],
                                start=(k == 0),
                                stop=(k == KH - 1),
                            )
                    tmp = sp.tile([128, GC, BL], F32, tag=f"tmp{dn}")
                    nc.vector.tensor_add(
                        tmp[:, 0:12, :], ptA[:], gx[(dn, layer)][:, 0:12, :, pos]
                    )
                    sig = sp.tile([128, GC, BL], F32, tag=f"sig{dn}")
                    nc.scalar.activation(sig[:, 0:8, :], tmp[:, 0:8, :], AF.Sigmoid)
                    nc.scalar.activation(sig[:, 8:12, :], tmp[:, 8:12, :], AF.Tanh)
                    c_old = cst[(dn, t % 2)]
                    c_new = cst[(dn, 1 - t % 2)]
                    ig = sp.tile([128, KH, BL], F32, tag=f"ig{dn}")
                    nc.vector.tensor_mul(ig[:], sig[:, 0:4, :], sig[:, 8:12, :])
                    nc.vector.tensor_mul(c_new[:], sig[:, 4:8, :], c_old[:])
                    nc.vector.tensor_add(c_new[:], c_new[:], ig[:])
                    th = sp.tile([128, KH, BL], F32, tag=f"th{dn}")
                    nc.scalar.activation(th[:], c_new[:], AF.Tanh)
                    nc.vector.tensor_add(
                        tmp[:, 12:16, :], ptB[:], gx[(dn, layer)][:, 12:16, :, pos]
                    )
                    nc.scalar.activation(sig[:, 12:16, :], tmp[:, 12:16, :], AF.Sigmoid)
                    nc.vector.tensor_mul(hh[:, :, ws, :], sig[:, 12:16, :], th[:])

        lstm_layer(0, steps)

        # ---- L1 input GEMM ------------------------------------------------------
        gx[("f", 1)] = pp.tile([128, GC, BL, S], BF16, tag="gxf", name="gx1f")
        gx[("b", 1)] = pp.tile([128, GC, BL, S], BF16, tag="gxb", name="gx1b")
        for dd, dn in enumerate("fb"):
            wih1_sb = wp.tile([128, 8, G], BF16, tag="wih")
            for k in range(8):
                dma(wih1_sb[:, k, :], d["wih1T"][dd, k])
            for c in range(GC):
                for b in range(BL):
                    pt = ps.tile([128, 256], F32, tag="mm")
                    for k in range(8):
                        rhs = (
                            hist[(0, "f")][:, k, 1 : S + 1, b]
                            if k < KH
                            else hist[(0, "b")][:, k - KH, 0:S, b]
                        )
                        nc.tensor.matmul(
                            pt[:],
                            wih1_sb[:, k, bass.ts(c, 128)],
                            rhs,
                            start=(k == 0),
                            stop=(k == 7),
                        )
                    nc.vector.tensor_scalar(
                        out=gx[(dn, 1)][:, c, b, :],
                        in0=pt[:],
                        scalar1=b1v(dd, c),
                        scalar2=None,
                        op0=ALU.add,
                    )

        for dd, dn in enumerate("fb"):
            whh_sb[dn] = wp.tile([128, 4, G], BF16, tag=f"whh{dn}", name=f"whh1{dn}")
            for k in range(4):
                dma(whh_sb[dn][:, k, :], d["whh1T"][dd, k])
        lstm_layer(1, steps)

        # ---- highway + fc -------------------------------------------------------
        hw_sb = wp.tile([128, 2, 8, 2 * H], BF16, tag="wih")
        for w in range(2):
            for k in range(8):
                dma(hw_sb[:, w, k, :], d["hwT"][w, k])

        def x1_slice(k, b):
            if k < KH:
                return hist[(1, "f")][:, k, 1 : S + 1, b]
            return hist[(1, "b")][:, k - KH, 0:S, b]

        x2 = pp.tile([128, 8, TOK], BF16, tag="gxf")
        for c in range(8):
            for b in range(BL):
                ptt = ps.tile([128, 256], F32, tag="mm")
                pth = ps.tile([128, 256], F32, tag="mm")
                for k in range(8):
                    nc.tensor.matmul(
                        ptt[:], hw_sb[:, 0, k, bass.ts(c, 128)], x1_slice(k, b),
                        start=(k == 0), stop=(k == 7),
                    )
                for k in range(8):
                    nc.tensor.matmul(
                        pth[:], hw_sb[:, 1, k, bass.ts(c, 128)], x1_slice(k, b),
                        start=(k == 0), stop=(k == 7),
                    )
                tg = sp.tile([128, 256], F32, tag="tg")
                nc.scalar.activation(tg[:], ptt[:], AF.Sigmoid, bias=hwbv(0, c))
                rl = sp.tile([128, 256], F32, tag="rl")
                nc.scalar.activation(rl[:], pth[:], AF.Relu, bias=hwbv(1, c))
                dd_ = sp.tile([128, 256], F32, tag="dd")
                nc.vector.tensor_sub(dd_[:], rl[:], x1_slice(c, b))
                nc.vector.tensor_mul(dd_[:], tg[:], dd_[:])
                nc.vector.tensor_add(x2[:, c, bass.ts(b, 256)], dd_[:], x1_slice(c, b))

        logits = pp.tile([T, BL, S], F32, tag="embX")
        for b in range(BL):
            pt = ps.tile([128, 256], F32, tag="mm")
            for k in range(8):
                nc.tensor.matmul(
                    pt[:T, :], fcw_sb[:, k, :], x2[:, k, bass.ts(b, 256)],
                    start=(k == 0), stop=(k == 7),
                )
            nc.scalar.activation(logits[:, b, :], pt[:T, :], AF.Identity, bias=fcb_sb)

        # ---- CRF ---------------------------------------------------------------
        expEm = pp.tile([T, BL, S], F32, tag="XT")
        nc.scalar.activation(expEm[:], logits[:], AF.Exp)
        expT = pp.tile([T, T], F32, tag="expT")
        nc.scalar.activation(expT[:], trans_sb, AF.Exp)
        expS = pp.tile([T, 1], F32, tag="expS")
        nc.scalar.activation(expS[:], svec_sb, AF.Exp)
        expE = pp.tile([T, 1], F32, tag="expE")
        nc.scalar.activation(expE[:], evec_sb, AF.Exp)

        afin = pp.tile([T, BL], F32, tag="afin")
        lacc = {}
        for ch in range(2):
            for par in range(2):
                lacc[(ch, par)] = pp.tile(
                    [1, 2], F32, tag=f"lacc{ch}{par}", name=f"lacc{ch}{par}"
                )
            nc.vector.memset(lacc[(ch, 0)][:], 0.0)
        ap = ctx.enter_context(tc.tile_pool(name="crf", bufs=4))

        # two independent 2-sequence scan chains, interleaved per step
        A = {}
        for ch in range(2):
            sl = slice(2 * ch, 2 * ch + 2)
            A[ch] = ap.tile([T, 2], F32, tag=f"A{ch}", name=f"A{ch}")
            nc.vector.tensor_scalar(
                out=A[ch][:], in0=expEm[:, sl, 0], scalar1=expS[:, 0:1],
                scalar2=None, op0=ALU.mult,
            )
        nren_seen = 0
        for t in range(1, steps):
            for ch in range(2):
                sl = slice(2 * ch, 2 * ch + 2)
                pt = ps.tile([128, 2], F32, tag="mm")
                nc.tensor.matmul(pt[:T, :], expT[:], A[ch][:], start=True, stop=True)
                A[ch] = ap.tile([T, 2], F32, tag=f"A{ch}", name=f"A{ch}")
                nc.vector.tensor_mul(A[ch][:], pt[:T, :], expEm[:, sl, t])
            if t % RENORM == 0:
                for ch in range(2):
                    sl = slice(2 * ch, 2 * ch + 2)
                    psS = ps.tile([1, 512], F32, tag="small")
                    nc.tensor.matmul(
                        psS[:, :2], ones_t[:], A[ch][:], start=True, stop=True
                    )
                    Ssb = ap.tile([1, 2], F32, tag=f"Ssb{ch}", name=f"Ssb{ch}")
                    nc.vector.tensor_copy(Ssb[:], psS[:, :2])
                    Sr = ap.tile([1, 2], F32, tag=f"Sr{ch}", name=f"Sr{ch}")
                    nc.vector.reciprocal(Sr[:], Ssb[:])
                    pB = ps.tile([128, 2], F32, tag="mm")
                    nc.tensor.matmul(pB[:T, :], ones_1t[:], Sr[:], start=True, stop=True)
                    A2 = ap.tile([T, 2], F32, tag=f"A{ch}", name=f"A{ch}")
                    nc.vector.tensor_mul(A2[:], A[ch][:], pB[:T, :])
                    A[ch] = A2
                    lnS = ap.tile([1, 2], F32, tag=f"lnS{ch}", name=f"lnS{ch}")
                    nc.scalar.activation(lnS[:], Ssb[:], AF.Ln)
                    nc.vector.tensor_mul(lnS[:], lnS[:], mren_sb[:, nren_seen, sl])
                    old, new = lacc[(ch, nren_seen % 2)], lacc[(ch, 1 - nren_seen % 2)]
                    nc.vector.tensor_add(new[:], old[:], lnS[:])
                nren_seen += 1
            if t >= min(S // 2 - 1, steps - 1):
                for ch in range(2):
                    sl = slice(2 * ch, 2 * ch + 2)
                    nc.vector.copy_predicated(
                        afin[:, sl], msel_sb[:, sl, t], A[ch][:]
                    )

        # logZ = ln(sum_j afin*expE) + lacc
        ae = op.tile([T, BL], F32, tag="ae")
        nc.vector.tensor_scalar(
            out=ae[:], in0=afin[:], scalar1=expE[:, 0:1], scalar2=None, op0=ALU.mult
        )
        psZ = ps.tile([1, 512], F32, tag="small")
        nc.tensor.matmul(psZ[:, :BL], ones_t[:], ae[:], start=True, stop=True)
        logZ = sp.tile([1, BL], F32, tag="logZ")
        nc.scalar.activation(logZ[:], psZ[:, :BL], AF.Ln)
        for ch in range(2):
            sl = slice(2 * ch, 2 * ch + 2)
            nc.vector.tensor_add(
                logZ[:, sl], logZ[:, sl], lacc[(ch, nren_seen % 2)][:]
            )

        # ---- numerator ----------------------------------------------------------
        emm = op.tile([T, BL, S], F32, tag="emm")
        nc.vector.tensor_mul(emm[:], logits[:], oh_sb)
        empart = sp.tile([T, BL], F32, tag="empart")
        nc.vector.reduce_sum(empart[:], emm[:], axis=AX.X)
        nv = sp.tile([T, BL], F32, tag="nv")
        nc.vector.tensor_scalar(
            out=nv[:], in0=s0e_sb[:, 0:BL], scalar1=svec_sb, scalar2=None,
            op0=ALU.mult,
        )
        ev = sp.tile([T, BL], F32, tag="ev")
        nc.vector.tensor_scalar(
            out=ev[:], in0=s0e_sb[:, BL : 2 * BL], scalar1=evec_sb, scalar2=None,
            op0=ALU.mult,
        )
        nc.vector.tensor_add(nv[:], nv[:], ev[:])
        nc.vector.tensor_add(nv[:], nv[:], empart[:])
        for b in range(BL):
            trp = op.tile([T, T], F32, tag="trp")
            nc.vector.tensor_mul(trp[:], aux17_sb[:, cp_base + T * b : cp_base + T * (b + 1)], trans_sb)
            trr = sp.tile([T, 1], F32, tag="trr")
            nc.vector.reduce_sum(trr[:], trp[:], axis=AX.X)
            nc.vector.tensor_add(nv[:, b : b + 1], nv[:, b : b + 1], trr[:])
        psN = ps.tile([1, 512], F32, tag="small")
        nc.tensor.matmul(psN[:, :BL], ones_t[:], nv[:], start=True, stop=True)
        num_sb = sp.tile([1, BL], F32, tag="num")
        nc.vector.tensor_copy(num_sb[:], psN[:, :BL])

        # ---- aux CE -------------------------------------------------------------
        psE = ps.tile([1, 512], F32, tag="small")
        lse = op.tile([1, BL, S], F32, tag="lse")
        for hlf in range(2):
            nc.tensor.matmul(
                psE[:, :512],
                ones_t[:],
                expEm[:, 2 * hlf : 2 * hlf + 2, :],
                start=True,
                stop=True,
            )
            nc.scalar.activation(
                lse[:, 2 * hlf : 2 * hlf + 2, :],
                psE[:].rearrange("o (b s) -> o b s", b=2),
                AF.Ln,
            )
        nc.vector.tensor_mul(lse[:], lse[:], vm_sb)
        lsum = sp.tile([1, BL], F32, tag="lsum")
        nc.vector.reduce_sum(lsum[:], lse[:], axis=AX.X)
        psM = ps.tile([1, 512], F32, tag="small")
        nc.tensor.matmul(psM[:, :BL], ones_t[:], empart[:], start=True, stop=True)
        aux_sb = sp.tile([1, BL], F32, tag="aux")
        nc.vector.tensor_sub(aux_sb[:], lsum[:], psM[:, :BL])

        dma(out_d[0:1, :], num_sb[:])
        dma(out_d[1:2, :], logZ[:])
        dma(out_d[2:3, :], aux_sb[:])

    nc.compile()
    return nc


def _prep_maps(inputs):
    bf = ml_dtypes.bfloat16
    x = np.asarray(inputs["x"]).astype(np.int32)
    tags = np.asarray(inputs["tags"]).astype(np.int32)
    emb = np.asarray(inputs["emb"], np.float32)

    def t2(w):  # (2, G, K) -> (2, K//128, 128, G)
        w = np.asarray(w, np.float32)
        K = w.shape[2]
        return np.ascontiguousarray(
            w.transpose(0, 2, 1).reshape(2, K // 128, 128, G)
        ).astype(bf)

    wih0 = t2(inputs["w_ih_l0"])
    whh0 = t2(inputs["w_hh_l0"])
    wih1 = t2(inputs["w_ih_l1"])
    whh1 = t2(inputs["w_hh_l1"])
    b0 = np.asarray(inputs["b_l0"], np.float32).reshape(2, GC, 128)
    b1 = np.asarray(inputs["b_l1"], np.float32).reshape(2, GC, 128)
    aux128 = np.zeros((128, 80), np.float32)
    aux128[:, 0:32] = b0.reshape(32, 128).T
    aux128[:, 32:64] = b1.reshape(32, 128).T
    hwT = np.stack(
        [
            np.asarray(inputs["hw_t_w"], np.float32).T.reshape(8, 128, 2 * H),
            np.asarray(inputs["hw_h_w"], np.float32).T.reshape(8, 128, 2 * H),
        ]
    ).astype(bf)
    aux128[:, 64:72] = np.asarray(inputs["hw_t_b"], np.float32).reshape(8, 128).T
    aux128[:, 72:80] = np.asarray(inputs["hw_h_b"], np.float32).reshape(8, 128).T
    fcwT = np.ascontiguousarray(
        np.asarray(inputs["fc_w"], np.float32).T.reshape(8, 128, T).transpose(1, 0, 2)
    ).reshape(128, 8 * T).astype(bf)
    trans = np.asarray(inputs["crf_trans"], np.float32)
    svec = np.asarray(inputs["crf_start"], np.float32)
    evec = np.asarray(inputs["crf_end"], np.float32)
    fcb = np.asarray(inputs["fc_b"], np.float32)

    valid = tags != 0
    lengths = (x != 0).sum(1)

    maps = []
    for c in range(NC):
        sl = slice(c * BL, (c + 1) * BL)
        xl, tl, vl, ll = x[sl], tags[sl], valid[sl], lengths[sl]
        flat = xl.reshape(-1)  # 256*b + t
        x_idx = np.ascontiguousarray(flat.reshape(TOK // 128, 128).T).astype(np.int32)
        jj = np.arange(T)
        oh = (tl[None, :, :] == jj[:, None, None]) & vl[None, :, :]
        oh_tags = oh.reshape(T, TOK).astype(np.float32)
        cp = np.zeros((T, BL, T), np.float32)
        for b in range(BL):
            for t in range(1, S):
                if vl[b, t]:
                    cp[tl[b, t - 1], b, tl[b, t]] += 1.0
        s0e = np.zeros((T, 2 * BL), np.float32)
        for b in range(BL):
            s0e[tl[b, 0], b] = 1.0
            s0e[tl[b, ll[b] - 1], BL + b] = 1.0
        msel = np.zeros((BL, S), np.float32)
        for b in range(BL):
            msel[b, ll[b] - 1] = 1.0
        msel = np.broadcast_to(msel.reshape(1, TOK), (T, TOK)).astype(np.uint8)
        mren = np.zeros((NREN, BL), np.float32)
        for k in range(NREN):
            mren[k] = (RENORM * (k + 1) <= ll - 1).astype(np.float32)
        aux17 = np.zeros((T, 1120), np.float32)
        aux17[:, 0:T] = trans
        aux17[:, T] = svec
        aux17[:, T + 1] = evec
        aux17[:, T + 2] = fcb
        aux17[:, 20 : 20 + TOK] = oh_tags
        aux17[:, 20 + TOK : 20 + TOK + BL * T] = cp.transpose(0, 1, 2).reshape(T, BL * T)
        aux17[:, 20 + TOK + BL * T : 20 + TOK + BL * T + 2 * BL] = s0e
        aux1 = np.concatenate(
            [mren.reshape(-1), vl.reshape(-1).astype(np.float32)]
        ).reshape(1, -1)
        maps.append(
            dict(
                x_idx=x_idx,
                emb=emb,
                wih0T=wih0,
                whh0T=whh0,
                wih1T=wih1,
                whh1T=whh1,
                hwT=hwT,
                fcwT=fcwT,
                aux128=aux128,
                aux17=aux17,
                aux1=aux1,
                msel=msel,
            )
        )
    return maps, valid


TRACE = {}


def kernel(**inputs):
    if "nc" not in _CACHE:
        _CACHE["nc"] = _build_nc()
    nc = _CACHE["nc"]
    maps, valid = _prep_maps(inputs)
    kw = {}
    if TRACE.get("on"):
        kw = dict(trace=True, tmpdir=TRACE.get("dir"), trace_cores=[0])
    res = run_bass_kernel_spmd(nc, maps, list(range(NC)), **kw)
    TRACE["last"] = res
    outs = [res.results[i]["out"] for i in range(NC)]
    num = np.concatenate([o[0] for o in outs])
    logZ = np.concatenate([o[1] for o in outs])
    aux = np.concatenate([o[2] for o in outs])
    crf_loss = -np.mean(num - logZ, dtype=np.float32)
    aux_loss = np.float32(aux.sum()) / np.float32(max(valid.sum(), 1))
    return np.float32(crf_loss + np.float32(0.1) * aux_loss)
